# revision 1
# baseline (speedup 1.0000x reference)
"""GNN message-passing kernel for TRN2 (8 NeuronCores, SPMD) — v2.

Math (see reference):
  h = relu(x @ W_in + b_in);  h = LayerNorm(h) * ln_g + ln_b
  deg/dinv from edge_src;  hn = h / (||h|| + 1e-4)
  for 3 layers:
     ang_i = sum_{e: src=i} dinv_src*dinv_dst*<hn_src, hn_dst>
     rotate hn[:,0:2] by ang (Givens)
  z = relu(h @ cW1 + cb1); bn-affine; logits = z @ cW2 + cb2; log_softmax

Algebraic restructuring (as v1):
  - Givens preserves ||h||; only hn[:,0:2] changes across layers.
  - ang_i = <hn_i, M_i>, M_i = sum_e w_e * hn_dst  (w_e = dinv_src*dinv_dst)
  - T_i (tail, dims 2:512) constant across layers; per-layer head part
    uses fresh (a,b)=hn[:,0:2]:  ang_i = T_i + a_i*P_i + b_i*Q_i.

v2 distribution/layout changes vs v1:
  - Phase 0 computes ONLY own nodes (6272/core); the normalized features
    are AllGathered in fp8 (scaled by 8) instead of being recomputed
    8x redundantly on every core.
  - The AllGather OUTPUT BUFFER IS the gather table: node (r,g,p) lives
    at 512B row (r*128+p)*GPC+g.  The per-layer (a,b) AllGather uses the
    same row indexing with 256B rows ([GPC,64] f32 padded).  One set of
    int16 gather-index tables serves phase 3 and both layer gathers;
    class y = (dst core >= 4) splits rows into two halves for int16.
  - Phase-3 segment-sum matmuls run in fp8 DoubleRow (256-edge blocks).
  - Classifier matmuls run in f32r.
"""

import math
import numpy as np
import ml_dtypes

import sys as _sys
for _p in ("/opt/trn_rl_repo", "/root/.axon_site/_ro/trn_rl_repo"):
    if _p not in _sys.path:
        _sys.path.insert(0, _p)
import concourse.bacc as bacc
import concourse.tile as tile
import concourse.bass as bass
import concourse.mybir as mybir
from concourse.masks import make_identity

dt = mybir.dt
P = 128
D = 512
DOUT = 40
NC = 8
LN_EPS = 1e-5
BN_EPS = 1e-5
NRM_EPS = 1e-4
Y8 = 8.0          # fp8 feature prescale


class Cfg:
    def __init__(self, n_cores, gpc, B, flags, g3=2, gl=4, vb=4):
        self.NC = n_cores
        self.GPC = gpc                   # groups (of 128 nodes) per core
        self.NPC = gpc * P               # nodes per core
        self.NPAD = n_cores * self.NPC
        self.ROWS = n_cores * P * gpc    # table rows (== NPAD)
        self.HALF_ROWS = self.ROWS // 2
        self.B = B                       # dict ycls -> blocks per group
        self.BT = B[0] + B[1]
        self.G3 = g3                     # phase-3 gather group batch
        self.GL = gl                     # layer gather group batch
        self.VB = vb                     # phase-0 block batch
        self.flags = flags


# ---------------------------------------------------------------- host prep

def host_prep(x, edge_src, edge_dst, n_cores=8, gpc=None):
    """Build per-core inputs + slot/index arrays. Returns (cfg, percore)."""
    N = x.shape[0]
    if gpc is None:
        gpc = (N + n_cores * P - 1) // (n_cores * P)
    NPC = gpc * P
    HALF_ROWS = n_cores * P * gpc // 2

    deg = np.bincount(edge_src, minlength=N).astype(np.float64)
    dinv = np.where(deg > 0, deg ** -0.5, 0.0).astype(np.float32)
    w_all = dinv[edge_src] * dinv[edge_dst]          # per-edge weight

    # table row of a node: (r*128 + p)*gpc + g
    def node_row(n):
        r = n // NPC
        nn = n % NPC
        g = nn // P
        p = nn % P
        return (r * P + p) * gpc + g

    src_core = edge_src // NPC
    percore_raw = []
    counts_all = np.zeros((n_cores, gpc, 2), np.int64)
    for r in range(n_cores):
        m = src_core == r
        es = edge_src[m]
        ww = w_all[m]
        rows = node_row(edge_dst[m].astype(np.int64))
        g = (es - r * NPC) // P
        ycls = (rows >= HALF_ROWS).astype(np.int64)
        key = (g * 2 + ycls).astype(np.int64)
        order = np.argsort(key, kind="stable")
        es, ww, rows, ycls = (a[order] for a in (es, ww, rows, ycls))
        counts_all[r] = np.bincount(key, minlength=gpc * 2).reshape(gpc, 2)
        percore_raw.append((es, ww, rows, ycls))

    kmax = counts_all.reshape(-1, 2).max(axis=0)
    # blocks per class: pad to 128 and round up to EVEN (DoubleRow pairs)
    B = {}
    for y in (0, 1):
        b = max(1, int((kmax[y] + P - 1) // P))
        B[y] = b + (b % 2)
    BT = B[0] + B[1]
    nslc = np.array([B[0] * P, B[1] * P], np.int64)
    slot_off = np.array([0, nslc[0]], np.int64)
    tot_slots = int(nslc.sum())

    xpad = np.zeros((n_cores * NPC, x.shape[1]), np.float32)
    xpad[:N] = x

    def wrap16(a2):      # [gpc, nslots] int16 -> [gpc, 128, nslots/16]
        w3 = a2.reshape(gpc, -1, 16).transpose(0, 2, 1)
        return np.ascontiguousarray(np.tile(w3, (1, 8, 1)))

    def slots_t(a2, s0, s1, nb):
        return a2[:, s0:s1].reshape(gpc, nb, P).transpose(0, 2, 1)

    percore = []
    for r in range(n_cores):
        es, ww, rows, ycls = percore_raw[r]
        cnt = counts_all[r]

        flat_starts = (np.arange(gpc)[:, None] * tot_slots + slot_off[None, :])
        csum = np.concatenate([[0], np.cumsum(cnt.reshape(-1))])[:-1].reshape(gpc, 2)
        e_idx = np.arange(len(es))
        bucket = ((es - r * NPC) // P) * 2 + ycls
        rank = e_idx - csum.reshape(-1)[bucket]
        slot = flat_starts.reshape(-1)[bucket] + rank

        srclf = np.full(gpc * tot_slots, -1.0, np.float32)
        yvf = np.zeros(gpc * tot_slots, np.int16)
        srclf[slot] = (es % P).astype(np.float32)
        yvf[slot] = (rows - ycls * HALF_ROWS).astype(np.int16)

        sf = srclf.reshape(gpc, tot_slots)
        yf = yvf.reshape(gpc, tot_slots)
        srcl = np.full((gpc, P, BT), -1.0, np.float32)
        yidx = {}
        boff = 0
        for y in (0, 1):
            s0, s1, nb = slot_off[y], slot_off[y] + nslc[y], B[y]
            srcl[:, :, boff:boff + nb] = slots_t(sf, s0, s1, nb)
            yidx[y] = wrap16(yf[:, s0:s1])
            boff += nb

        dinv_own = np.ascontiguousarray(
            dinv[np.arange(r * NPC, (r + 1) * NPC) % N].reshape(gpc, P)
            * (np.arange(r * NPC, (r + 1) * NPC) < N).reshape(gpc, P))
        xT_own = np.ascontiguousarray(xpad[r * NPC:(r + 1) * NPC].T)
        percore.append(dict(xT=xT_own,
                            srcl=srcl.astype(np.float32),
                            dinv=dinv_own.astype(np.float32),
                            yidx0=yidx[0], yidx1=yidx[1]))

    cfg = Cfg(n_cores, gpc, B, {})
    return cfg, percore


# ---------------------------------------------------------------- device build

def build_nc(cfg, skip_cc=False):
    GPC, NPC, ROWS, HALF_ROWS = cfg.GPC, cfg.NPC, cfg.ROWS, cfg.HALF_ROWS
    B, BT, G3, GL, VB = cfg.B, cfg.BT, cfg.G3, cfg.GL, cfg.VB
    FL = cfg.flags

    f32, f32r, bf16, i16 = dt.float32, dt.float32r, dt.bfloat16, dt.int16
    f8 = dt.float8e4
    AF = mybir.ActivationFunctionType
    OP = mybir.AluOpType
    DR = mybir.MatmulPerfMode.DoubleRow

    nc = bacc.Bacc("TRN2", target_bir_lowering=False, debug=False, num_devices=NC)

    # ---------------- I/O ----------------
    xT = nc.dram_tensor("xT", [D, NPC], f32, kind="ExternalInput").ap()
    W_in = nc.dram_tensor("W_in", [D, D], f32, kind="ExternalInput").ap()
    b_in = nc.dram_tensor("b_in", [1, D], f32, kind="ExternalInput").ap()
    ln_g = nc.dram_tensor("ln_g", [1, D], f32, kind="ExternalInput").ap()
    ln_b = nc.dram_tensor("ln_b", [1, D], f32, kind="ExternalInput").ap()
    cW1 = nc.dram_tensor("cW1", [D, D], f32, kind="ExternalInput").ap()
    cb1 = nc.dram_tensor("cb1", [1, D], f32, kind="ExternalInput").ap()
    bn_g = nc.dram_tensor("bn_g", [1, D], f32, kind="ExternalInput").ap()
    bn_b = nc.dram_tensor("bn_b", [1, D], f32, kind="ExternalInput").ap()
    bn_m = nc.dram_tensor("bn_m", [1, D], f32, kind="ExternalInput").ap()
    bn_v = nc.dram_tensor("bn_v", [1, D], f32, kind="ExternalInput").ap()
    cW2 = nc.dram_tensor("cW2", [D, DOUT], f32, kind="ExternalInput").ap()
    cb2 = nc.dram_tensor("cb2", [1, DOUT], f32, kind="ExternalInput").ap()
    srclT = nc.dram_tensor("srcl", [GPC, P, BT], f32, kind="ExternalInput").ap()
    dinvT = nc.dram_tensor("dinv", [GPC, P], f32, kind="ExternalInput").ap()
    yidxT = {}
    for y in (0, 1):
        yidxT[y] = nc.dram_tensor(f"yidx{y}", [GPC, P, B[y] * 8], i16,
                                  kind="ExternalInput").ap()
    out = nc.dram_tensor("out", [NPC, DOUT], f32, kind="ExternalOutput").ap()

    # ---------------- internal DRAM ----------------
    hn_own = nc.dram_tensor("hn_own", [NPC, D], bf16, kind="Internal").ap()

    from contextlib import ExitStack
    with tile.TileContext(nc) as tc, ExitStack() as stack:
        pers = stack.enter_context(tc.tile_pool(name="pers", bufs=1))
        dram = stack.enter_context(tc.tile_pool(name="dram", bufs=1, space="DRAM"))
        dram2 = stack.enter_context(tc.tile_pool(name="dram2", bufs=2, space="DRAM"))

        # collective buffers (DRAM); the OUT buffers are the gather tables
        ccy_in = dram.tile([P, GPC * D], f8, tag="ccy_in")
        ccy_out = dram.tile([NC, P, GPC * D], f8, tag="ccy_out")

        # persistent tiles
        w_in_sb = pers.tile([P, 4, D], f32r)
        cw1b = pers.tile([P, 4, D], bf16)
        cw2b = pers.tile([P, 4, DOUT], bf16)
        iota_f = pers.tile([P, P], bf16)
        ident = pers.tile([P, P], f32)
        halfpi = pers.tile([P, 1], f32)
        epsln = pers.tile([P, 1], f32)
        epsbn1 = pers.tile([1, 1], f32)
        amT = pers.tile([P, 4], f32)     # bn alpha, of-major [p, k]
        bmT = pers.tile([P, 4], f32)     # bn beta
        cb1mT = pers.tile([P, 4], f32)
        epsb_p = pers.tile([P, 1], f32)
        gml = pers.tile([P, D], f32)     # ln gamma mat (general path)
        bml = pers.tile([P, D], f32)
        dsc_t = pers.tile([P, GPC], f32)  # dinv/(8*(||h||+eps))
        a_own = pers.tile([P, GPC], f32)
        b_own = pers.tile([P, GPC], f32)
        d_own = pers.tile([P, GPC], f32)
        T_own = pers.tile([P, GPC], f32)
        ang1 = pers.tile([P, GPC], f32)
        P_all = pers.tile([P, GPC], f32)
        Q_all = pers.tile([P, GPC], f32)
        c_t = pers.tile([P, GPC], f32)
        s_t = pers.tile([P, GPC], f32)
        r1 = pers.tile([P, GPC], f32)
        r2 = pers.tile([P, GPC], f32)
        r3 = pers.tile([P, GPC], f32)
        r4 = pers.tile([P, GPC], f32)
        angL = pers.tile([P, GPC], f32)
        uvp64 = pers.tile([P, GPC, 64], f32)
        srcl_all = pers.tile([P, GPC, BT], f32)
        dinv_sb = pers.tile([P, GPC], f32)
        e8d = pers.tile([P, GPC], f32)       # 8 * dinv
        bnt2 = pers.tile([1, D], f32)
        binm = pers.tile([P, D], f32)
        cb2m = pers.tile([P, DOUT], f32)

        # ---- one-time setup ----
        nc.sync.dma_start(out=w_in_sb[:], in_=W_in.rearrange(
            "(k p) f -> p k f", k=4, p=P).bitcast(f32r))
        nc.gpsimd.dma_start(out=cw1b[:], in_=cW1.rearrange(
            "(k p) f -> p k f", k=4, p=P))
        nc.gpsimd.dma_start(out=cw2b[:], in_=cW2.rearrange(
            "(k p) f -> p k f", k=4, p=P))
        nc.sync.dma_start(out=srcl_all[:], in_=srclT.rearrange("g p s -> p g s"))
        nc.sync.dma_start(out=dinv_sb[:], in_=dinvT.rearrange("g p -> p g"))
        nc.vector.tensor_scalar_mul(out=e8d[:], in0=dinv_sb[:], scalar1=Y8)
        nc.gpsimd.memset(halfpi[:], math.pi / 2)
        nc.gpsimd.memset(epsln[:], LN_EPS)
        nc.gpsimd.memset(epsbn1[:], BN_EPS)
        nc.gpsimd.memset(uvp64[:], 0.0)
        iota_i = pers.tile([P, P], dt.int32)
        nc.gpsimd.iota(iota_i[:], pattern=[[1, P]], base=0, channel_multiplier=0)
        nc.vector.tensor_copy(out=iota_f[:], in_=iota_i[:])
        make_identity(nc, ident[:])

        # bn alpha/beta in of-major [P, 4] layout (of = k*128 + p)
        nc.gpsimd.memset(epsb_p[:], BN_EPS)
        bnv4 = pers.tile([P, 4], f32)
        bng4 = pers.tile([P, 4], f32)
        bnm4 = pers.tile([P, 4], f32)
        bnb4 = pers.tile([P, 4], f32)
        nc.sync.dma_start(out=bnv4[:], in_=bn_v.rearrange("o (k p) -> p (o k)", k=4, p=P))
        nc.sync.dma_start(out=bng4[:], in_=bn_g.rearrange("o (k p) -> p (o k)", k=4, p=P))
        nc.sync.dma_start(out=bnm4[:], in_=bn_m.rearrange("o (k p) -> p (o k)", k=4, p=P))
        nc.sync.dma_start(out=bnb4[:], in_=bn_b.rearrange("o (k p) -> p (o k)", k=4, p=P))
        nc.scalar.activation(bnv4[:], bnv4[:], AF.Sqrt, bias=epsb_p[:])
        nc.vector.reciprocal(out=bnv4[:], in_=bnv4[:])
        nc.vector.tensor_mul(out=amT[:], in0=bng4[:], in1=bnv4[:])
        nc.vector.tensor_mul(out=bnm4[:], in0=bnm4[:], in1=amT[:])
        nc.vector.tensor_sub(out=bmT[:], in0=bnb4[:], in1=bnm4[:])
        if not FL.get("ln_trivial", False):
            nc.sync.dma_start(out=bnt2[:], in_=ln_g[:])
            nc.gpsimd.partition_broadcast(gml[:], bnt2[:])
            nc.sync.dma_start(out=bnt2[:], in_=ln_b[:])
            nc.gpsimd.partition_broadcast(bml[:], bnt2[:])
        if not FL.get("bin_zero", True):
            nc.sync.dma_start(out=bnt2[:], in_=b_in[:])
            nc.gpsimd.partition_broadcast(binm[:], bnt2[:])
        if not FL.get("cb1_zero", True):
            nc.sync.dma_start(out=cb1mT[:], in_=cb1.rearrange("o (k p) -> p (o k)", k=4, p=P))
        if not FL.get("cb2_zero", True):
            bnt6 = pers.tile([1, DOUT], f32)
            nc.sync.dma_start(out=bnt6[:], in_=cb2[:])
            nc.gpsimd.partition_broadcast(cb2m[:], bnt6[:])

        # seq builder: one-hot(src%128) per slot, bf16 [P, BT, P].
        # One tensor_scalar per slot-block (2-dim APs keep the HW verifier
        # happy and hit the DVE fast path); w_e is factored out (dinv folded
        # into tables and per-partition scales).
        def build_seq(pool, g, tag="selw"):
            seq = pool.tile([P, BT, P], bf16, tag=tag)
            for b in range(BT):
                nc.vector.tensor_scalar(
                    out=seq[:, b, :], in0=iota_f[:],
                    scalar1=srcl_all[:, g, b:b + 1], scalar2=None,
                    op0=OP.is_equal)
            return seq

        # ================= phase 0: dense + LN + normalize (own nodes) ====
        ccy_in_v = ccy_in[:].rearrange("p (g e) -> p g e", g=GPC, e=D)
        with tc.tile_pool(name="p0", bufs=2) as p0, \
             tc.tile_pool(name="p0ps", bufs=3, space="PSUM") as p0ps:
            n_batches = GPC // VB + (1 if GPC % VB else 0)
            for mb in range(n_batches):
                v0 = mb * VB
                nv = min(VB, GPC - v0)
                xb = p0.tile([P, VB, 4, P], f32r, tag="xb")
                xTr = xT.rearrange("(k p) (b n) -> p b k n", k=4, p=P, n=P)
                for v in range(nv):
                    nc.sync.dma_start(out=xb[:, v], in_=xTr[:, v0 + v].bitcast(f32r))
                hnb = p0.tile([P, VB, D], f32, tag="hnb")
                mu_s = p0.tile([P, VB], f32, tag="mu_s")
                var_s = p0.tile([P, VB], f32, tag="var_s")
                sd_t = p0.tile([P, VB], f32, tag="sd_t")
                istd = p0.tile([P, VB], f32, tag="istd")
                sv_t = p0.tile([P, VB], f32, tag="sv_t")
                nrm_t = p0.tile([P, VB], f32, tag="nrm_t")
                dba = p0.tile([P, VB], f32, tag="dba")
                idv = p0.tile([P, VB], f32, tag="idv")
                sc_t = p0.tile([P, VB], f32, tag="sc_t")
                hsb = []
                hcb = []
                for v in range(nv):
                    ph = p0ps.tile([P, D], f32, tag="ph", space="PSUM")
                    for k in range(4):
                        nc.tensor.matmul(out=ph[:], lhsT=xb[:, v, k, :],
                                         rhs=w_in_sb[:, k, :],
                                         start=(k == 0), stop=(k == 3))
                    h_sb = p0.tile([P, D], f32, tag=f"h{v}")
                    if not FL.get("bin_zero", True):
                        nc.vector.tensor_add(out=h_sb[:], in0=ph[:], in1=binm[:])
                        nc.vector.tensor_scalar_max(out=h_sb[:], in0=h_sb[:], scalar1=0.0)
                    else:
                        nc.vector.tensor_scalar_max(out=h_sb[:], in0=ph[:], scalar1=0.0)
                    nc.vector.reduce_sum(out=mu_s[:, v:v + 1], in_=h_sb[:],
                                         axis=mybir.AxisListType.X)
                    hsb.append(h_sb)
                nc.vector.tensor_scalar_mul(out=mu_s[:, 0:nv], in0=mu_s[:, 0:nv],
                                            scalar1=-1.0 / D)
                for v in range(nv):
                    hc = p0.tile([P, D], f32, tag=f"hc{v}")
                    nc.vector.tensor_scalar_add(out=hc[:], in0=hsb[v][:],
                                                scalar1=mu_s[:, v:v + 1])
                    sq = p0.tile([P, D], f32, tag="sq")
                    nc.scalar.activation(sq[:], hc[:], AF.Square,
                                         accum_out=var_s[:, v:v + 1])
                    hcb.append(hc)
                hf = p0.tile([P, VB, D], bf16, tag="hf")
                if FL.get("ln_trivial", True):
                    nc.scalar.activation(sd_t[:, 0:nv], var_s[:, 0:nv], AF.Sqrt,
                                         bias=epsln[:], scale=1.0 / D)
                    nc.vector.reciprocal(out=istd[:, 0:nv], in_=sd_t[:, 0:nv])
                    nc.scalar.activation(sv_t[:, 0:nv], var_s[:, 0:nv], AF.Sqrt)
                    nc.vector.tensor_mul(out=nrm_t[:, 0:nv], in0=istd[:, 0:nv],
                                         in1=sv_t[:, 0:nv])
                    nc.vector.tensor_scalar_add(out=dba[:, 0:nv], in0=nrm_t[:, 0:nv],
                                                scalar1=NRM_EPS)
                    nc.vector.reciprocal(out=idv[:, 0:nv], in_=dba[:, 0:nv])
                    nc.vector.tensor_mul(out=sc_t[:, 0:nv], in0=istd[:, 0:nv],
                                         in1=idv[:, 0:nv])
                    for v in range(nv):
                        nc.scalar.activation(hnb[:, v, :], hcb[v][:], AF.Copy,
                                             scale=sc_t[:, v:v + 1])
                        nc.scalar.activation(hf[:, v, :], hcb[v][:], AF.Copy,
                                             scale=istd[:, v:v + 1])
                    nc.vector.tensor_mul(out=dsc_t[:, v0:v0 + nv],
                                         in0=dinv_sb[:, v0:v0 + nv],
                                         in1=idv[:, 0:nv])
                else:
                    nc.scalar.activation(sd_t[:, 0:nv], var_s[:, 0:nv], AF.Sqrt,
                                         bias=epsln[:], scale=1.0 / D)
                    nc.vector.reciprocal(out=istd[:, 0:nv], in_=sd_t[:, 0:nv])
                    for v in range(nv):
                        hl = p0.tile([P, D], f32, tag=f"hl{v}")
                        nc.vector.scalar_tensor_tensor(
                            out=hl[:], in0=hcb[v][:], scalar=istd[:, v:v + 1],
                            in1=gml[:], op0=OP.mult, op1=OP.mult)
                        nc.vector.tensor_add(out=hl[:], in0=hl[:], in1=bml[:])
                        sq2 = p0.tile([P, D], f32, tag="sq")
                        nc.vector.scalar_tensor_tensor(
                            out=sq2[:], in0=hl[:], scalar=1.0, in1=hl[:],
                            op0=OP.mult, op1=OP.mult,
                            accum_out=nrm_t[:, v:v + 1])
                        hcb[v] = hl
                    nc.scalar.activation(sv_t[:, 0:nv], nrm_t[:, 0:nv], AF.Sqrt)
                    nc.vector.tensor_scalar_add(out=dba[:, 0:nv], in0=sv_t[:, 0:nv],
                                                scalar1=NRM_EPS)
                    nc.vector.reciprocal(out=sc_t[:, 0:nv], in_=dba[:, 0:nv])
                    for v in range(nv):
                        nc.scalar.activation(hnb[:, v, :], hcb[v][:], AF.Copy,
                                             scale=sc_t[:, v:v + 1])
                        nc.scalar.activation(hf[:, v, :], hcb[v][:], AF.Copy)
                    nc.vector.tensor_mul(out=dsc_t[:, v0:v0 + nv],
                                         in0=dinv_sb[:, v0:v0 + nv],
                                         in1=sc_t[:, 0:nv])
                # fp8 features (scaled by 8*dinv) -> collective input table
                y8 = p0.tile([P, VB, D], f8, tag="y8")
                for v in range(nv):
                    nc.vector.tensor_scalar_mul(
                        out=y8[:, v, :], in0=hnb[:, v, :],
                        scalar1=e8d[:, v0 + v:v0 + v + 1])
                nc.gpsimd.dma_start(out=ccy_in_v[:, v0:v0 + nv, :],
                                    in_=y8[:, 0:nv, :])
                nc.sync.dma_start(
                    out=hn_own[v0 * P:(v0 + nv) * P, :].rearrange(
                        "(v p) e -> p v e", v=nv, p=P),
                    in_=hf[:, 0:nv, :])
                for v in range(nv):
                    m = v0 + v
                    nc.vector.tensor_copy(out=a_own[:, m:m + 1], in_=hnb[:, v, 0:1])
                    nc.vector.tensor_copy(out=b_own[:, m:m + 1], in_=hnb[:, v, 1:2])
                    nc.vector.tensor_copy(out=d_own[:, m:m + 1], in_=dba[:, v:v + 1])

        nc.vector.tensor_scalar_mul(out=dsc_t[:], in0=dsc_t[:], scalar1=1.0 / Y8)

        # AllGather the fp8 feature table
        if not skip_cc:
            nc.gpsimd.collective_compute(
                "AllGather", mybir.AluOpType.bypass,
                replica_groups=[list(range(NC))],
                ins=[ccy_in.opt()], outs=[ccy_out.opt()])
        else:
            nc.gpsimd.dma_start(out=ccy_out[0], in_=ccy_in[:])

        yrows = ccy_out[:].rearrange("r p (g e) -> (r p g) e", g=GPC, e=D)

        # ================= phase 3: full-dot aggregation =================
        with tc.tile_pool(name="p3", bufs=2) as p3, \
             tc.tile_pool(name="p3ps", bufs=2, space="PSUM") as p3ps:
            n3 = GPC // G3 + (1 if GPC % G3 else 0)
            for ib in range(n3):
                g0 = ib * G3
                gn = min(G3, GPC - g0)
                tg = {}
                for y in (0, 1):
                    s = B[y] * 8
                    tidx = p3.tile([P, G3 * s], i16, tag=f"yi{y}")
                    nc.sync.dma_start(
                        out=tidx[:, 0:gn * s].rearrange("p (g s) -> p g s",
                                                        g=gn, s=s),
                        in_=yidxT[y][g0:g0 + gn].rearrange("g p s -> p g s"))
                    t = p3.tile([P, G3 * B[y], D], f8, tag=f"tg{y}")
                    nc.gpsimd.dma_gather(
                        out_ap=t[:, 0:gn * B[y], :],
                        in_ap=yrows[y * HALF_ROWS:(y + 1) * HALF_ROWS, :],
                        idxs_ap=tidx[:, 0:gn * s],
                        num_idxs=gn * B[y] * P,
                        num_idxs_reg=gn * B[y] * P, elem_size=D,
                        single_packet=False)
                    tg[y] = t
                for gi in range(gn):
                    g = g0 + gi
                    seq = build_seq(p3, g)
                    seq8 = p3.tile([P, BT, P], f8, tag="seq8")
                    nc.scalar.activation(seq8[:], seq[:], AF.Copy)
                    pm = p3ps.tile([P, D], f32, tag="M", space="PSUM")
                    nmm = BT // 2
                    mi = 0
                    for y in (0, 1):
                        for db in range(B[y] // 2):
                            boff = (0 if y == 0 else B[0]) + 2 * db
                            j0 = gi * B[y] + 2 * db
                            nc.tensor.matmul(
                                out=pm[:],
                                lhsT=seq8[:, boff:boff + 2, :],
                                rhs=tg[y][:, j0:j0 + 2, :],
                                start=(mi == 0), stop=(mi == nmm - 1),
                                perf_mode=DR)
                            mi += 1
                    hsl = p3.tile([P, D], bf16, tag="hsl")
                    nc.sync.dma_start(out=hsl[:], in_=hn_own[g * P:(g + 1) * P, :])
                    hs = p3.tile([P, D], f32, tag="hs")
                    nc.scalar.activation(hs[:], hsl[:], AF.Copy,
                                         scale=dsc_t[:, g:g + 1])
                    scr = p3.tile([P, D], f32, tag="scr")
                    nc.vector.tensor_mul(out=scr[:], in0=pm[:], in1=hs[:])
                    nc.vector.reduce_sum(out=ang1[:, g:g + 1], in_=scr[:],
                                         axis=mybir.AxisListType.X)
                    hsum = p3.tile([P, 1], f32, tag="hsum")
                    nc.vector.reduce_sum(out=hsum[:], in_=scr[:, 0:2],
                                         axis=mybir.AxisListType.X)
                    nc.vector.tensor_sub(out=T_own[:, g:g + 1],
                                         in0=ang1[:, g:g + 1], in1=hsum[:])

        # ================= layers =================
        for layer in (1, 2, 3):
            if layer == 1:
                ang_src = ang1
            else:
                ccab_in = dram2.tile([P, GPC * 64], f32, tag="ccab_in")
                ccab_out = dram2.tile([NC, P, GPC * 64], f32, tag="ccab_out")
                nc.vector.tensor_mul(out=uvp64[:, :, 0:1], in0=a_own[:, :, None],
                                     in1=dinv_sb[:, :, None])
                nc.vector.tensor_mul(out=uvp64[:, :, 1:2], in0=b_own[:, :, None],
                                     in1=dinv_sb[:, :, None])
                nc.gpsimd.dma_start(
                    out=ccab_in[:], in_=uvp64[:].rearrange("p g e -> p (g e)"))
                if not skip_cc:
                    nc.gpsimd.collective_compute(
                        "AllGather", mybir.AluOpType.bypass,
                        replica_groups=[list(range(NC))],
                        ins=[ccab_in.opt()], outs=[ccab_out.opt()])
                else:
                    nc.gpsimd.dma_start(out=ccab_out[0], in_=ccab_in[:])
                abrows = ccab_out[:].rearrange("r p (g e) -> (r p g) e",
                                               g=GPC, e=64)
                with tc.tile_pool(name=f"l{layer}", bufs=2) as lp, \
                     tc.tile_pool(name=f"l{layer}ps", bufs=2, space="PSUM") as lps:
                    nbat = GPC // GL + (1 if GPC % GL else 0)
                    for ib in range(nbat):
                        g0 = ib * GL
                        gn = min(GL, GPC - g0)
                        tab = {}
                        for y in (0, 1):
                            s = B[y] * 8
                            tidx = lp.tile([P, GL * s], i16, tag=f"ui{y}")
                            nc.sync.dma_start(
                                out=tidx[:, 0:gn * s].rearrange(
                                    "p (g s) -> p g s", g=gn, s=s),
                                in_=yidxT[y][g0:g0 + gn].rearrange("g p s -> p g s"))
                            t = lp.tile([P, GL * B[y], 64], f32, tag=f"tu{y}")
                            nc.gpsimd.dma_gather(
                                out_ap=t[:, 0:gn * B[y], :],
                                in_ap=abrows[y * HALF_ROWS:(y + 1) * HALF_ROWS, :],
                                idxs_ap=tidx[:, 0:gn * s],
                                num_idxs=gn * B[y] * P,
                                num_idxs_reg=gn * B[y] * P, elem_size=64,
                                single_packet=False)
                            tab[y] = t
                        for gi in range(gn):
                            g = g0 + gi
                            seq = build_seq(lp, g)
                            uvc = lp.tile([P, BT, 2], bf16, tag="uvc")
                            for y in (0, 1):
                                boff = 0 if y == 0 else B[0]
                                nc.vector.tensor_copy(
                                    out=uvc[:, boff:boff + B[y], :],
                                    in_=tab[y][:, gi * B[y]:(gi + 1) * B[y], 0:2])
                            pq = lps.tile([P, 2], f32, tag="PQ", space="PSUM")
                            for bg in range(BT):
                                nc.tensor.matmul(
                                    out=pq[:], lhsT=seq[:, bg, :],
                                    rhs=uvc[:, bg, :],
                                    start=(bg == 0), stop=(bg == BT - 1))
                            nc.vector.tensor_copy(out=P_all[:, g:g + 1],
                                                  in_=pq[:, 0:1])
                            nc.vector.tensor_copy(out=Q_all[:, g:g + 1],
                                                  in_=pq[:, 1:2])
                nc.vector.tensor_mul(out=r1[:], in0=P_all[:], in1=a_own[:])
                nc.vector.tensor_mul(out=r2[:], in0=Q_all[:], in1=b_own[:])
                nc.vector.tensor_add(out=r1[:], in0=r1[:], in1=r2[:])
                nc.vector.tensor_mul(out=r1[:], in0=r1[:], in1=dinv_sb[:])
                nc.vector.tensor_add(out=angL[:], in0=T_own[:], in1=r1[:])
                ang_src = angL
            nc.scalar.activation(c_t[:], ang_src[:], AF.Sin, bias=halfpi[:])
            nc.scalar.activation(s_t[:], ang_src[:], AF.Sin)
            nc.vector.tensor_mul(out=r1[:], in0=c_t[:], in1=a_own[:])
            nc.vector.tensor_mul(out=r2[:], in0=s_t[:], in1=b_own[:])
            nc.vector.tensor_mul(out=r3[:], in0=s_t[:], in1=a_own[:])
            nc.vector.tensor_mul(out=r4[:], in0=c_t[:], in1=b_own[:])
            nc.vector.tensor_sub(out=a_own[:], in0=r1[:], in1=r2[:])
            nc.vector.tensor_add(out=b_own[:], in0=r3[:], in1=r4[:])

        # ---- write final (a,b)*d into hn_own cols 0:2 (h head values) ----
        nc.vector.tensor_mul(out=r1[:], in0=a_own[:], in1=d_own[:])
        nc.vector.tensor_mul(out=r2[:], in0=b_own[:], in1=d_own[:])
        abw = pers.tile([P, GPC, 2], bf16)
        hTall = pers.tile([P, 4, NPC], bf16)
        nc.vector.tensor_copy(out=abw[:, :, 0:1], in_=r1[:, :, None])
        nc.vector.tensor_copy(out=abw[:, :, 1:2], in_=r2[:, :, None])
        nc.sync.dma_start(
            out=hn_own.rearrange("(g p) e -> p g e", g=GPC, p=P)[:, :, 0:2],
            in_=abw[:])
        for k in range(4):
            nc.sync.dma_start_transpose(out=hTall[:, k, :],
                                        in_=hn_own[:, k * P:(k + 1) * P])

        # ================= phase 5: classifier (transposed domain) ========
        with tc.tile_pool(name="p5", bufs=3) as p5, \
             tc.tile_pool(name="p5ps", bufs=2, space="PSUM") as p5ps:
            for g in range(GPC):
                zTd = p5ps.tile([P, 4, P], f32, tag="zT", space="PSUM")
                for of in range(4):
                    for k in range(4):
                        nc.tensor.matmul(out=zTd[:, of, :],
                                         lhsT=cw1b[:, k, of * P:(of + 1) * P],
                                         rhs=hTall[:, k, g * P:(g + 1) * P],
                                         start=(k == 0), stop=(k == 3))
                zr = p5.tile([P, 4, P], bf16, tag="zr")
                if not FL.get("cb1_zero", True):
                    for of in range(4):
                        nc.vector.tensor_scalar(
                            out=zr[:, of, :], in0=zTd[:, of, :],
                            scalar1=cb1mT[:, of:of + 1], scalar2=0.0,
                            op0=OP.add, op1=OP.max)
                else:
                    nc.vector.tensor_scalar_max(out=zr[:], in0=zTd[:], scalar1=0.0)
                z2 = p5.tile([P, 4, P], bf16, tag="z2")
                for of in range(4):
                    nc.vector.tensor_scalar(
                        out=z2[:, of, :], in0=zr[:, of, :],
                        scalar1=amT[:, of:of + 1], scalar2=bmT[:, of:of + 1],
                        op0=OP.mult, op1=OP.add)
                lgT = p5ps.tile([DOUT, P], f32, tag="lgps", space="PSUM")
                for k in range(4):
                    nc.tensor.matmul(out=lgT[:], lhsT=cw2b[:, k, :],
                                     rhs=z2[:, k, :],
                                     start=(k == 0), stop=(k == 3))
                lg_sb = p5.tile([DOUT, P], f32, tag="lgsb")
                nc.scalar.activation(lg_sb[:], lgT[:], AF.Copy)
                ptr = p5ps.tile([P, DOUT], f32, tag="tr", space="PSUM")
                nc.tensor.transpose(out=ptr[:], in_=lg_sb[:],
                                    identity=ident[0:DOUT, 0:DOUT])
                lgv = p5.tile([P, DOUT], f32, tag="lgv")
                if not FL.get("cb2_zero", True):
                    nc.vector.tensor_add(out=lgv[:], in0=ptr[:], in1=cb2m[:])
                else:
                    nc.vector.tensor_copy(out=lgv[:], in_=ptr[:])
                mx = p5.tile([P, 1], f32, tag="mx")
                nc.vector.reduce_max(out=mx[:], in_=lgv[:], axis=mybir.AxisListType.X)
                sh = p5.tile([P, DOUT], f32, tag="sh")
                nc.vector.tensor_scalar_sub(out=sh[:], in0=lgv[:], scalar1=mx[:])
                ex = p5.tile([P, DOUT], f32, tag="ex")
                se = p5.tile([P, 1], f32, tag="se")
                nc.scalar.activation(ex[:], sh[:], AF.Exp, accum_out=se[:])
                ls = p5.tile([P, 1], f32, tag="ls")
                nc.scalar.activation(ls[:], se[:], AF.Ln)
                ob = p5.tile([P, DOUT], f32, tag="ob")
                nc.vector.tensor_scalar_sub(out=ob[:], in0=sh[:], scalar1=ls[:])
                nc.sync.dma_start(out=out[g * P:(g + 1) * P, :], in_=ob[:])

    nc.compile()
    return nc


# ---------------------------------------------------------------- in_maps

def make_in_maps(cfg, percore, weights):
    ins = []
    for r in range(cfg.NC):
        pc = percore[r]
        m = dict(
            xT=pc["xT"],
            W_in=weights["W_in"], b_in=weights["b_in"][None, :],
            ln_g=weights["ln_g"][None, :], ln_b=weights["ln_b"][None, :],
            cW1=weights["cW1"], cb1=weights["cb1"][None, :],
            bn_g=weights["bn_g"][None, :], bn_b=weights["bn_b"][None, :],
            bn_m=weights["bn_mean"][None, :], bn_v=weights["bn_var"][None, :],
            cW2=weights["cW2"], cb2=weights["cb2"][None, :],
            srcl=pc["srcl"], dinv=pc["dinv"],
            yidx0=pc["yidx0"], yidx1=pc["yidx1"],
        )
        ins.append(m)
    return ins


def assemble_output(cfg, results, n):
    chunks = [results[r]["out"] for r in range(cfg.NC)]
    full = np.concatenate(chunks, axis=0)
    return full[:n]


# ---------------------------------------------------------------- entry point

def _cfg_flags(w):
    return dict(
        bin_zero=bool(np.all(w["b_in"] == 0)),
        ln_trivial=bool(np.all(w["ln_g"] == 1) and np.all(w["ln_b"] == 0)),
        cb1_zero=bool(np.all(w["cb1"] == 0)),
        cb2_zero=bool(np.all(w["cb2"] == 0)),
    )


def kernel(**inputs):
    """Full-input GNN forward on 8 TRN2 NeuronCores; returns [N, 40] fp32."""
    x = np.asarray(inputs["x"], np.float32)
    edge_src = np.asarray(inputs["edge_src"])
    edge_dst = np.asarray(inputs["edge_dst"])
    w = {k: np.asarray(inputs[k], np.float32) for k in
         ["W_in", "b_in", "ln_g", "ln_b", "cW1", "cb1", "bn_g", "bn_b",
          "bn_mean", "bn_var", "cW2", "cb2"]}
    N = x.shape[0]

    cfg, percore = host_prep(x, edge_src, edge_dst, n_cores=8)
    cfg.flags = _cfg_flags(w)
    nc = build_nc(cfg)
    in_maps = make_in_maps(cfg, percore, w)

    from concourse.bass_utils import run_bass_kernel_spmd
    res = run_bass_kernel_spmd(nc, in_maps, core_ids=list(range(cfg.NC)))
    return assemble_output(cfg, res.results, N).astype(np.float32)


def estimate_exec_ns(inputs):
    """Tile cost-model (TimelineSim) estimate of the per-core program span.

    Collective latencies are excluded (replaced by local shard copies to
    preserve the dependency structure); everything else is modeled."""
    x = np.asarray(inputs["x"], np.float32)
    w = {k: np.asarray(inputs[k], np.float32) for k in
         ["W_in", "b_in", "ln_g", "ln_b", "cW1", "cb1", "bn_g", "bn_b",
          "bn_mean", "bn_var", "cW2", "cb2"]}
    cfg, _ = host_prep(x, np.asarray(inputs["edge_src"]),
                       np.asarray(inputs["edge_dst"]), n_cores=8)
    cfg.flags = _cfg_flags(w)
    nc2 = build_nc(cfg, skip_cc=True)
    from concourse.timeline_sim import TimelineSim
    tl = TimelineSim(nc2, trace=False)
    ns = tl.simulate()
    return int(ns)



# revision 19
# speedup vs baseline: 1.2970x; 1.2970x over previous
"""GNN message-passing kernel for TRN2 (8 NeuronCores, SPMD) — v2.

Math (see reference):
  h = relu(x @ W_in + b_in);  h = LayerNorm(h) * ln_g + ln_b
  deg/dinv from edge_src;  hn = h / (||h|| + 1e-4)
  for 3 layers:
     ang_i = sum_{e: src=i} dinv_src*dinv_dst*<hn_src, hn_dst>
     rotate hn[:,0:2] by ang (Givens)
  z = relu(h @ cW1 + cb1); bn-affine; logits = z @ cW2 + cb2; log_softmax

Algebraic restructuring (as v1):
  - Givens preserves ||h||; only hn[:,0:2] changes across layers.
  - ang_i = <hn_i, M_i>, M_i = sum_e w_e * hn_dst  (w_e = dinv_src*dinv_dst)
  - T_i (tail, dims 2:512) constant across layers; per-layer head part
    uses fresh (a,b)=hn[:,0:2]:  ang_i = T_i + a_i*P_i + b_i*Q_i.

v2 distribution/layout changes vs v1:
  - Phase 0 computes ONLY own nodes (6272/core); the normalized features
    are AllGathered in fp8 (scaled by 8) instead of being recomputed
    8x redundantly on every core.
  - The AllGather OUTPUT BUFFER IS the gather table: node (r,g,p) lives
    at 512B row (r*128+p)*GPC+g.  The per-layer (a,b) AllGather uses the
    same row indexing with 256B rows ([GPC,64] f32 padded).  One set of
    int16 gather-index tables serves phase 3 and both layer gathers;
    class y = (dst core >= 4) splits rows into two halves for int16.
  - Phase-3 segment-sum matmuls run in fp8 DoubleRow (256-edge blocks).
  - Classifier matmuls run in f32r.
"""

import math
import numpy as np
import ml_dtypes

import sys as _sys
for _p in ("/opt/trn_rl_repo", "/root/.axon_site/_ro/trn_rl_repo"):
    if _p not in _sys.path:
        _sys.path.insert(0, _p)
import concourse.bacc as bacc
import concourse.tile as tile
import concourse.bass as bass
import concourse.mybir as mybir
from concourse.masks import make_identity

dt = mybir.dt
P = 128
D = 512
DOUT = 40
NC = 8
LN_EPS = 1e-5
BN_EPS = 1e-5
NRM_EPS = 1e-4
Y8 = 8.0          # fp8 feature prescale


class Cfg:
    def __init__(self, n_cores, gpc, B, flags, g3=2, vb=4, lkg=None):
        self.NC = n_cores
        self.GPC = gpc                   # groups (of 128 nodes) per core
        self.NPC = gpc * P               # nodes per core
        self.NPAD = n_cores * self.NPC
        self.ROWS = n_cores * P * gpc    # table rows (== NPAD)
        self.HALF_ROWS = self.ROWS // 2
        self.B = B                       # dict ycls -> blocks per group
        self.BT = B[0] + B[1]
        self.G3 = g3                     # phase-3 gather group batch
        self.VB = vb                     # phase-0 block batch
        self.LKG = lkg                   # per-(g, half) layer slots per node
        self.flags = flags


# ---------------------------------------------------------------- host prep

def host_prep(x, edge_src, edge_dst, n_cores=8, gpc=None):
    """Build per-core inputs + slot/index arrays. Returns (cfg, percore)."""
    N = x.shape[0]
    if gpc is None:
        gpc = (N + n_cores * P - 1) // (n_cores * P)
    NPC = gpc * P
    HALF_ROWS = n_cores * P * gpc // 2

    deg = np.bincount(edge_src, minlength=N).astype(np.float64)
    dinv = np.where(deg > 0, deg ** -0.5, 0.0).astype(np.float32)
    w_all = dinv[edge_src] * dinv[edge_dst]          # per-edge weight

    # table row of a node: (r*128 + p)*gpc + g
    def node_row(n):
        r = n // NPC
        nn = n % NPC
        g = nn // P
        p = nn % P
        return (r * P + p) * gpc + g

    src_core = edge_src // NPC
    percore_raw = []
    counts_all = np.zeros((n_cores, gpc, 2), np.int64)
    for r in range(n_cores):
        m = src_core == r
        es = edge_src[m]
        ww = w_all[m]
        rows = node_row(edge_dst[m].astype(np.int64))
        g = (es - r * NPC) // P
        ycls = (rows >= HALF_ROWS).astype(np.int64)
        key = (g * 2 + ycls).astype(np.int64)
        order = np.argsort(key, kind="stable")
        es, ww, rows, ycls = (a[order] for a in (es, ww, rows, ycls))
        counts_all[r] = np.bincount(key, minlength=gpc * 2).reshape(gpc, 2)
        percore_raw.append((es, ww, rows, ycls))

    kmax = counts_all.reshape(-1, 2).max(axis=0)
    # blocks per class: pad to 128 and round up to EVEN (DoubleRow pairs)
    B = {}
    for y in (0, 1):
        b = max(1, int((kmax[y] + P - 1) // P))
        B[y] = b + (b % 2)
    BT = B[0] + B[1]
    nslc = np.array([B[0] * P, B[1] * P], np.int64)
    slot_off = np.array([0, nslc[0]], np.int64)
    tot_slots = int(nslc.sum())

    xpad = np.zeros((n_cores * NPC, x.shape[1]), np.float32)
    xpad[:N] = x

    def wrap16(a2):      # [gpc, nslots] int16 -> [gpc, 128, nslots/16]
        w3 = a2.reshape(gpc, -1, 16).transpose(0, 2, 1)
        return np.ascontiguousarray(np.tile(w3, (1, 8, 1)))

    def slots_t(a2, s0, s1, nb):
        return a2[:, s0:s1].reshape(gpc, nb, P).transpose(0, 2, 1)

    percore = []
    for r in range(n_cores):
        es, ww, rows, ycls = percore_raw[r]
        cnt = counts_all[r]

        flat_starts = (np.arange(gpc)[:, None] * tot_slots + slot_off[None, :])
        csum = np.concatenate([[0], np.cumsum(cnt.reshape(-1))])[:-1].reshape(gpc, 2)
        e_idx = np.arange(len(es))
        bucket = ((es - r * NPC) // P) * 2 + ycls
        rank = e_idx - csum.reshape(-1)[bucket]
        slot = flat_starts.reshape(-1)[bucket] + rank

        srclf = np.full(gpc * tot_slots, -1.0, np.float32)
        yvf = np.zeros(gpc * tot_slots, np.int16)
        srclf[slot] = (es % P).astype(np.float32)
        yvf[slot] = (rows - ycls * HALF_ROWS).astype(np.int16)

        sf = srclf.reshape(gpc, tot_slots)
        yf = yvf.reshape(gpc, tot_slots)
        srcl = np.full((gpc, P, BT), -1.0, np.float32)
        yidx = {}
        boff = 0
        for y in (0, 1):
            s0, s1, nb = slot_off[y], slot_off[y] + nslc[y], B[y]
            srcl[:, :, boff:boff + nb] = slots_t(sf, s0, s1, nb)
            yidx[y] = wrap16(yf[:, s0:s1])
            boff += nb

        dinv_own = np.ascontiguousarray(
            dinv[np.arange(r * NPC, (r + 1) * NPC) % N].reshape(gpc, P)
            * (np.arange(r * NPC, (r + 1) * NPC) < N).reshape(gpc, P))
        xT_own = np.ascontiguousarray(xpad[r * NPC:(r + 1) * NPC].T)
        percore.append(dict(xT=xT_own,
                            srcl=srcl.astype(np.float32),
                            dinv=dinv_own.astype(np.float32),
                            yidx0=yidx[0], yidx1=yidx[1]))

    # ------- layer-gather tables (ap_gather from replicated half-tables) ---
    # Layers 2,3 fetch (a*dinv, b*dinv) per edge via GPSIMD ap_gather from a
    # [HALF_ROWS+1]-entry f32 half-table replicated across partitions (bf16
    # lanes pack the pair; entry HALF_ROWS is zero).  Per 16-partition group
    # G, half y, list position j = off(g)*16 + k*16 + pG holds the k-th
    # y-edge of node (16G+pG, g); k-major keeps the diagonal mask pattern
    # periodic in 32 elements independent of the per-(g, y) slot count K.
    Kg = np.zeros((gpc, 2), np.int64)
    pgk = []
    for r in range(n_cores):
        es, ww, rows, ycls = percore_raw[r]
        node = (es - r * NPC).astype(np.int64)
        key = node * 2 + ycls
        order = np.argsort(key, kind="stable")
        ks = np.empty(len(es), np.int64)
        csum = np.concatenate(
            [[0], np.cumsum(np.bincount(key, minlength=NPC * 2))])
        ks[order] = np.arange(len(es)) - csum[key[order]]
        cnt = np.bincount(key, minlength=NPC * 2).reshape(gpc, P, 2)
        Kg = np.maximum(Kg, cnt.max(axis=1))
        pgk.append((node % P, node // P, ks,
                    (rows - ycls * HALF_ROWS).astype(np.int64), ycls))
    Kg = Kg.astype(np.int64)
    offs = {y: np.concatenate([[0], np.cumsum(Kg[:, y])]) for y in (0, 1)}
    NIh = {y: 16 * int(offs[y][-1]) for y in (0, 1)}
    for r in range(n_cores):
        p, g, ks, loc, ycls = pgk[r]
        for y in (0, 1):
            m = ycls == y
            j = (offs[y][g[m]] + ks[m]) * 16 + (p[m] % 16)
            arr = np.full((P, NIh[y] // 16), HALF_ROWS, np.int16)
            arr[(p[m] // 16) * 16 + (j % 16), j // 16] = loc[m].astype(np.int16)
            percore[r][f"lidx{y}"] = np.ascontiguousarray(arr)
    Kmax = int(Kg.max())
    msk = np.zeros((P, Kmax, 16, 2), np.float32)
    for pp in range(P):
        msk[pp, :, pp % 16, :] = 1.0
    lmask = np.ascontiguousarray(
        msk.reshape(P, Kmax * 32)).astype(ml_dtypes.bfloat16)
    for r in range(n_cores):
        percore[r]["lmask"] = lmask

    cfg = Cfg(n_cores, gpc, B, {}, lkg=Kg)
    return cfg, percore


# ---------------------------------------------------------------- device build

def build_nc(cfg, skip_cc=False):
    GPC, NPC, ROWS, HALF_ROWS = cfg.GPC, cfg.NPC, cfg.ROWS, cfg.HALF_ROWS
    B, BT, G3, VB = cfg.B, cfg.BT, cfg.G3, cfg.VB
    Kg = cfg.LKG
    Kmax = int(Kg.max())
    LOFF = {y: np.concatenate([[0], np.cumsum(Kg[:, y])]).astype(int)
            for y in (0, 1)}
    NIh = {y: 16 * int(LOFF[y][-1]) for y in (0, 1)}
    FL = cfg.flags

    f32, f32r, bf16, i16 = dt.float32, dt.float32r, dt.bfloat16, dt.int16
    f8 = dt.float8e4
    AF = mybir.ActivationFunctionType
    OP = mybir.AluOpType
    DR = mybir.MatmulPerfMode.DoubleRow

    nc = bacc.Bacc("TRN2", target_bir_lowering=False, debug=False, num_devices=NC)

    # ---------------- I/O ----------------
    xT = nc.dram_tensor("xT", [D, NPC], f32, kind="ExternalInput").ap()
    W_in = nc.dram_tensor("W_in", [D, D], f32, kind="ExternalInput").ap()
    b_in = nc.dram_tensor("b_in", [1, D], f32, kind="ExternalInput").ap()
    ln_g = nc.dram_tensor("ln_g", [1, D], f32, kind="ExternalInput").ap()
    ln_b = nc.dram_tensor("ln_b", [1, D], f32, kind="ExternalInput").ap()
    cW1 = nc.dram_tensor("cW1", [D, D], f32, kind="ExternalInput").ap()
    cb1 = nc.dram_tensor("cb1", [1, D], f32, kind="ExternalInput").ap()
    bn_g = nc.dram_tensor("bn_g", [1, D], f32, kind="ExternalInput").ap()
    bn_b = nc.dram_tensor("bn_b", [1, D], f32, kind="ExternalInput").ap()
    bn_m = nc.dram_tensor("bn_m", [1, D], f32, kind="ExternalInput").ap()
    bn_v = nc.dram_tensor("bn_v", [1, D], f32, kind="ExternalInput").ap()
    cW2 = nc.dram_tensor("cW2", [D, DOUT], f32, kind="ExternalInput").ap()
    cb2 = nc.dram_tensor("cb2", [1, DOUT], f32, kind="ExternalInput").ap()
    srclT = nc.dram_tensor("srcl", [GPC, P, BT], f32, kind="ExternalInput").ap()
    dinvT = nc.dram_tensor("dinv", [GPC, P], f32, kind="ExternalInput").ap()
    yidxT = {}
    for y in (0, 1):
        yidxT[y] = nc.dram_tensor(f"yidx{y}", [GPC, P, B[y] * 8], i16,
                                  kind="ExternalInput").ap()
    lidxT = {}
    for y in (0, 1):
        lidxT[y] = nc.dram_tensor(f"lidx{y}", [P, NIh[y] // 16], i16,
                                  kind="ExternalInput").ap()
    lmaskT = nc.dram_tensor("lmask", [P, Kmax * 32], dt.bfloat16,
                            kind="ExternalInput").ap()
    out = nc.dram_tensor("out", [NPC, DOUT], f32, kind="ExternalOutput").ap()

    # ---------------- internal DRAM ----------------
    hn_own = nc.dram_tensor("hn_own", [NPC, D], bf16, kind="Internal").ap()

    from contextlib import ExitStack
    with tile.TileContext(nc) as tc, ExitStack() as stack:
        pers = stack.enter_context(tc.tile_pool(name="pers", bufs=1))
        dram = stack.enter_context(tc.tile_pool(name="dram", bufs=1, space="DRAM"))
        dram2 = stack.enter_context(tc.tile_pool(name="dram2", bufs=2, space="DRAM"))

        # collective buffers (DRAM); the OUT buffers are the gather tables
        ccy_in = dram.tile([P, GPC * D], f8, tag="ccy_in")
        ccy_out = dram.tile([NC, P, GPC * D], f8, tag="ccy_out")

        # persistent tiles
        halfpi = pers.tile([P, 1], f32)
        epsln = pers.tile([P, 1], f32)
        epsbn1 = pers.tile([1, 1], f32)
        amT = pers.tile([P, 4], f32)     # bn alpha, of-major [p, k]
        bmT = pers.tile([P, 4], f32)     # bn beta
        cb1mT = pers.tile([P, 4], f32)
        epsb_p = pers.tile([P, 1], f32)
        dsc_t = pers.tile([P, GPC], f32)  # dinv/(8*(||h||+eps))
        a_own = pers.tile([P, GPC], f32)
        b_own = pers.tile([P, GPC], f32)
        d_own = pers.tile([P, GPC], f32)
        T_own = pers.tile([P, GPC], f32)
        ang1 = pers.tile([P, GPC], f32)
        P_all = pers.tile([P, GPC], f32)
        Q_all = pers.tile([P, GPC], f32)
        c_t = pers.tile([P, GPC], f32)
        s_t = pers.tile([P, GPC], f32)
        r1 = pers.tile([P, GPC], f32)
        r2 = pers.tile([P, GPC], f32)
        r3 = pers.tile([P, GPC], f32)
        r4 = pers.tile([P, GPC], f32)
        angL = pers.tile([P, GPC], f32)
        dinv_sb = pers.tile([P, GPC], f32)
        e8d = pers.tile([P, GPC], f32)       # 8 * dinv
        cb2m = pers.tile([P, DOUT], f32)

        # ---- one-time setup ----
        nc.sync.dma_start(out=dinv_sb[:], in_=dinvT.rearrange("g p -> p g"))
        nc.vector.tensor_scalar_mul(out=e8d[:], in0=dinv_sb[:], scalar1=Y8)
        nc.gpsimd.memset(halfpi[:], math.pi / 2)
        nc.gpsimd.memset(epsln[:], LN_EPS)
        nc.gpsimd.memset(epsbn1[:], BN_EPS)

        # bn alpha/beta in of-major [P, 4] layout (of = k*128 + p)
        nc.gpsimd.memset(epsb_p[:], BN_EPS)
        bnv4 = pers.tile([P, 4], f32)
        bng4 = pers.tile([P, 4], f32)
        bnm4 = pers.tile([P, 4], f32)
        bnb4 = pers.tile([P, 4], f32)
        nc.sync.dma_start(out=bnv4[:], in_=bn_v.rearrange("o (k p) -> p (o k)", k=4, p=P))
        nc.sync.dma_start(out=bng4[:], in_=bn_g.rearrange("o (k p) -> p (o k)", k=4, p=P))
        nc.sync.dma_start(out=bnm4[:], in_=bn_m.rearrange("o (k p) -> p (o k)", k=4, p=P))
        nc.sync.dma_start(out=bnb4[:], in_=bn_b.rearrange("o (k p) -> p (o k)", k=4, p=P))
        nc.scalar.activation(bnv4[:], bnv4[:], AF.Sqrt, bias=epsb_p[:])
        nc.vector.reciprocal(out=bnv4[:], in_=bnv4[:])
        nc.vector.tensor_mul(out=amT[:], in0=bng4[:], in1=bnv4[:])
        nc.vector.tensor_mul(out=bnm4[:], in0=bnm4[:], in1=amT[:])
        nc.vector.tensor_sub(out=bmT[:], in0=bnb4[:], in1=bnm4[:])
        if not FL.get("cb1_zero", True):
            nc.sync.dma_start(out=cb1mT[:], in_=cb1.rearrange("o (k p) -> p (o k)", k=4, p=P))
        if not FL.get("cb2_zero", True):
            bnt6 = pers.tile([1, DOUT], f32)
            nc.sync.dma_start(out=bnt6[:], in_=cb2[:])
            nc.gpsimd.partition_broadcast(cb2m[:], bnt6[:])
        iota_f = None  # created in the phase-3 scope

        # seq builder: one-hot(src%128) per slot, bf16 [P, BT, P].
        # One tensor_scalar per slot-block (2-dim APs keep the HW verifier
        # happy and hit the DVE fast path); w_e is factored out (dinv folded
        # into tables and per-partition scales).
        def build_seq(pool, g, iota_f, srcl_all, tag="selw"):
            seq = pool.tile([P, BT, P], bf16, tag=tag)
            for b in range(BT):
                nc.vector.tensor_scalar(
                    out=seq[:, b, :], in0=iota_f[:],
                    scalar1=srcl_all[:, g, b:b + 1], scalar2=None,
                    op0=OP.is_equal)
            return seq

        # ================= phase 0: dense + LN + normalize (own nodes) ====
        ccy_in_v = ccy_in[:].rearrange("p (g e) -> p g e", g=GPC, e=D)
        with tc.tile_pool(name="ph0c", bufs=1) as ph0c, \
             tc.tile_pool(name="p0", bufs=2) as p0, \
             tc.tile_pool(name="p0ps", bufs=3, space="PSUM") as p0ps:
            w_in_sb = ph0c.tile([P, 4, D], f32r)
            nc.sync.dma_start(out=w_in_sb[:], in_=W_in.rearrange(
                "(k p) f -> p k f", k=4, p=P).bitcast(f32r))
            gml = ph0c.tile([P, D], f32)
            bml = ph0c.tile([P, D], f32)
            binm = ph0c.tile([P, D], f32)
            bnt2 = ph0c.tile([1, D], f32)
            if not FL.get("ln_trivial", False):
                nc.sync.dma_start(out=bnt2[:], in_=ln_g[:])
                nc.gpsimd.partition_broadcast(gml[:], bnt2[:])
                nc.sync.dma_start(out=bnt2[:], in_=ln_b[:])
                nc.gpsimd.partition_broadcast(bml[:], bnt2[:])
            if not FL.get("bin_zero", True):
                nc.sync.dma_start(out=bnt2[:], in_=b_in[:])
                nc.gpsimd.partition_broadcast(binm[:], bnt2[:])
            n_batches = GPC // VB + (1 if GPC % VB else 0)
            for mb in range(n_batches):
                v0 = mb * VB
                nv = min(VB, GPC - v0)
                xb = p0.tile([P, VB, 4, P], f32r, tag="xb")
                xTr = xT.rearrange("(k p) (b n) -> p b k n", k=4, p=P, n=P)
                for v in range(nv):
                    nc.sync.dma_start(out=xb[:, v], in_=xTr[:, v0 + v].bitcast(f32r))
                hnb = p0.tile([P, VB, D], f32, tag="hnb")
                mu_s = p0.tile([P, VB], f32, tag="mu_s")
                var_s = p0.tile([P, VB], f32, tag="var_s")
                sd_t = p0.tile([P, VB], f32, tag="sd_t")
                istd = p0.tile([P, VB], f32, tag="istd")
                sv_t = p0.tile([P, VB], f32, tag="sv_t")
                nrm_t = p0.tile([P, VB], f32, tag="nrm_t")
                dba = p0.tile([P, VB], f32, tag="dba")
                idv = p0.tile([P, VB], f32, tag="idv")
                sc_t = p0.tile([P, VB], f32, tag="sc_t")
                hsb = []
                hcb = []
                for v in range(nv):
                    ph = p0ps.tile([P, D], f32, tag="ph", space="PSUM")
                    for k in range(4):
                        nc.tensor.matmul(out=ph[:], lhsT=xb[:, v, k, :],
                                         rhs=w_in_sb[:, k, :],
                                         start=(k == 0), stop=(k == 3))
                    h_sb = p0.tile([P, D], f32, tag=f"h{v}")
                    if not FL.get("bin_zero", True):
                        nc.vector.tensor_add(out=h_sb[:], in0=ph[:], in1=binm[:])
                        nc.vector.tensor_scalar_max(out=h_sb[:], in0=h_sb[:], scalar1=0.0)
                    else:
                        nc.vector.tensor_scalar_max(out=h_sb[:], in0=ph[:], scalar1=0.0)
                    nc.vector.reduce_sum(out=mu_s[:, v:v + 1], in_=h_sb[:],
                                         axis=mybir.AxisListType.X)
                    hsb.append(h_sb)
                nc.vector.tensor_scalar_mul(out=mu_s[:, 0:nv], in0=mu_s[:, 0:nv],
                                            scalar1=-1.0 / D)
                for v in range(nv):
                    hc = p0.tile([P, D], f32, tag=f"hc{v}")
                    nc.vector.tensor_scalar_add(out=hc[:], in0=hsb[v][:],
                                                scalar1=mu_s[:, v:v + 1])
                    sq = p0.tile([P, D], f32, tag="sq")
                    nc.scalar.activation(sq[:], hc[:], AF.Square,
                                         accum_out=var_s[:, v:v + 1])
                    hcb.append(hc)
                hf = p0.tile([P, VB, D], bf16, tag="hf")
                if FL.get("ln_trivial", True):
                    nc.scalar.activation(sd_t[:, 0:nv], var_s[:, 0:nv], AF.Sqrt,
                                         bias=epsln[:], scale=1.0 / D)
                    nc.vector.reciprocal(out=istd[:, 0:nv], in_=sd_t[:, 0:nv])
                    nc.scalar.activation(sv_t[:, 0:nv], var_s[:, 0:nv], AF.Sqrt)
                    nc.vector.tensor_mul(out=nrm_t[:, 0:nv], in0=istd[:, 0:nv],
                                         in1=sv_t[:, 0:nv])
                    nc.vector.tensor_scalar_add(out=dba[:, 0:nv], in0=nrm_t[:, 0:nv],
                                                scalar1=NRM_EPS)
                    nc.vector.reciprocal(out=idv[:, 0:nv], in_=dba[:, 0:nv])
                    nc.vector.tensor_mul(out=sc_t[:, 0:nv], in0=istd[:, 0:nv],
                                         in1=idv[:, 0:nv])
                    for v in range(nv):
                        nc.scalar.activation(hnb[:, v, :], hcb[v][:], AF.Copy,
                                             scale=sc_t[:, v:v + 1])
                        nc.scalar.activation(hf[:, v, :], hcb[v][:], AF.Copy,
                                             scale=istd[:, v:v + 1])
                    nc.vector.tensor_mul(out=dsc_t[:, v0:v0 + nv],
                                         in0=dinv_sb[:, v0:v0 + nv],
                                         in1=idv[:, 0:nv])
                else:
                    nc.scalar.activation(sd_t[:, 0:nv], var_s[:, 0:nv], AF.Sqrt,
                                         bias=epsln[:], scale=1.0 / D)
                    nc.vector.reciprocal(out=istd[:, 0:nv], in_=sd_t[:, 0:nv])
                    for v in range(nv):
                        hl = p0.tile([P, D], f32, tag=f"hl{v}")
                        nc.vector.scalar_tensor_tensor(
                            out=hl[:], in0=hcb[v][:], scalar=istd[:, v:v + 1],
                            in1=gml[:], op0=OP.mult, op1=OP.mult)
                        nc.vector.tensor_add(out=hl[:], in0=hl[:], in1=bml[:])
                        sq2 = p0.tile([P, D], f32, tag="sq")
                        nc.vector.scalar_tensor_tensor(
                            out=sq2[:], in0=hl[:], scalar=1.0, in1=hl[:],
                            op0=OP.mult, op1=OP.mult,
                            accum_out=nrm_t[:, v:v + 1])
                        hcb[v] = hl
                    nc.scalar.activation(sv_t[:, 0:nv], nrm_t[:, 0:nv], AF.Sqrt)
                    nc.vector.tensor_scalar_add(out=dba[:, 0:nv], in0=sv_t[:, 0:nv],
                                                scalar1=NRM_EPS)
                    nc.vector.reciprocal(out=sc_t[:, 0:nv], in_=dba[:, 0:nv])
                    for v in range(nv):
                        nc.scalar.activation(hnb[:, v, :], hcb[v][:], AF.Copy,
                                             scale=sc_t[:, v:v + 1])
                        nc.scalar.activation(hf[:, v, :], hcb[v][:], AF.Copy)
                    nc.vector.tensor_mul(out=dsc_t[:, v0:v0 + nv],
                                         in0=dinv_sb[:, v0:v0 + nv],
                                         in1=sc_t[:, 0:nv])
                # fp8 features (scaled by 8*dinv) -> collective input table
                y8 = p0.tile([P, VB, D], f8, tag="y8")
                for v in range(nv):
                    nc.vector.tensor_scalar_mul(
                        out=y8[:, v, :], in0=hnb[:, v, :],
                        scalar1=e8d[:, v0 + v:v0 + v + 1])
                nc.gpsimd.dma_start(out=ccy_in_v[:, v0:v0 + nv, :],
                                    in_=y8[:, 0:nv, :])
                nc.sync.dma_start(
                    out=hn_own[v0 * P:(v0 + nv) * P, :].rearrange(
                        "(v p) e -> p v e", v=nv, p=P),
                    in_=hf[:, 0:nv, :])
                for v in range(nv):
                    m = v0 + v
                    nc.vector.tensor_copy(out=a_own[:, m:m + 1], in_=hnb[:, v, 0:1])
                    nc.vector.tensor_copy(out=b_own[:, m:m + 1], in_=hnb[:, v, 1:2])
                    nc.vector.tensor_copy(out=d_own[:, m:m + 1], in_=dba[:, v:v + 1])

        nc.vector.tensor_scalar_mul(out=dsc_t[:], in0=dsc_t[:], scalar1=1.0 / Y8)

        # AllGather the fp8 feature table
        if not skip_cc:
            nc.gpsimd.collective_compute(
                "AllGather", mybir.AluOpType.bypass,
                replica_groups=[list(range(NC))],
                ins=[ccy_in.opt()], outs=[ccy_out.opt()])
        else:
            nc.gpsimd.dma_start(out=ccy_out[0], in_=ccy_in[:])

        yrows = ccy_out[:].rearrange("r p (g e) -> (r p g) e", g=GPC, e=D)

        # ================= phase 3: full-dot aggregation =================
        with tc.tile_pool(name="p3c", bufs=1) as p3c, \
             tc.tile_pool(name="p3", bufs=2) as p3, \
             tc.tile_pool(name="p3ps", bufs=2, space="PSUM") as p3ps:
            srcl_all = p3c.tile([P, GPC, BT], f32)
            nc.sync.dma_start(out=srcl_all[:],
                              in_=srclT.rearrange("g p s -> p g s"))
            iota_i = p3c.tile([P, P], dt.int32)
            iota_f = p3c.tile([P, P], bf16)
            nc.gpsimd.iota(iota_i[:], pattern=[[1, P]], base=0,
                           channel_multiplier=0)
            nc.vector.tensor_copy(out=iota_f[:], in_=iota_i[:])
            n3 = GPC // G3 + (1 if GPC % G3 else 0)
            for ib in range(n3):
                g0 = ib * G3
                gn = min(G3, GPC - g0)
                tg = {}
                for y in (0, 1):
                    s = B[y] * 8
                    tidx = p3.tile([P, G3 * s], i16, tag=f"yi{y}")
                    nc.sync.dma_start(
                        out=tidx[:, 0:gn * s].rearrange("p (g s) -> p g s",
                                                        g=gn, s=s),
                        in_=yidxT[y][g0:g0 + gn].rearrange("g p s -> p g s"))
                    t = p3.tile([P, G3 * B[y], D], f8, tag=f"tg{y}")
                    nc.gpsimd.dma_gather(
                        out_ap=t[:, 0:gn * B[y], :],
                        in_ap=yrows[y * HALF_ROWS:(y + 1) * HALF_ROWS, :],
                        idxs_ap=tidx[:, 0:gn * s],
                        num_idxs=gn * B[y] * P,
                        num_idxs_reg=gn * B[y] * P, elem_size=D,
                        single_packet=False)
                    tg[y] = t
                for gi in range(gn):
                    g = g0 + gi
                    seq = build_seq(p3, g, iota_f, srcl_all)
                    seq8 = p3.tile([P, BT, P], f8, tag="seq8")
                    nc.scalar.activation(seq8[:], seq[:], AF.Copy)
                    pm = p3ps.tile([P, D], f32, tag="M", space="PSUM")
                    nmm = BT // 2
                    mi = 0
                    for y in (0, 1):
                        for db in range(B[y] // 2):
                            boff = (0 if y == 0 else B[0]) + 2 * db
                            j0 = gi * B[y] + 2 * db
                            nc.tensor.matmul(
                                out=pm[:],
                                lhsT=seq8[:, boff:boff + 2, :],
                                rhs=tg[y][:, j0:j0 + 2, :],
                                start=(mi == 0), stop=(mi == nmm - 1),
                                perf_mode=DR)
                            mi += 1
                    hsl = p3.tile([P, D], bf16, tag="hsl")
                    nc.sync.dma_start(out=hsl[:], in_=hn_own[g * P:(g + 1) * P, :])
                    hs = p3.tile([P, D], f32, tag="hs")
                    nc.scalar.activation(hs[:], hsl[:], AF.Copy,
                                         scale=dsc_t[:, g:g + 1])
                    scr = p3.tile([P, D], f32, tag="scr")
                    nc.vector.tensor_mul(out=scr[:], in0=pm[:], in1=hs[:])
                    nc.vector.reduce_sum(out=ang1[:, g:g + 1], in_=scr[:],
                                         axis=mybir.AxisListType.X)
                    hsum = p3.tile([P, 1], f32, tag="hsum")
                    nc.vector.reduce_sum(out=hsum[:], in_=scr[:, 0:2],
                                         axis=mybir.AxisListType.X)
                    nc.vector.tensor_sub(out=T_own[:, g:g + 1],
                                         in0=ang1[:, g:g + 1], in1=hsum[:])

        # ================= layers =================
        # Layers 2,3: P_i = sum_e (a*dinv)_dst and Q_i likewise are fetched
        # per edge by ONE GPSIMD ap_gather per half from a partition-
        # replicated [HALF_ROWS+1]-entry f32 table (bf16 lanes pack the
        # pair; last entry is zero for padding slots).  k-major list order
        # makes the diagonal mask (pG == p%16) periodic in 32 elements, so
        # one [P, Kmax*32] mask serves every per-(g, half) slot count K.
        # Consume per (g, half): masked mul (DVE 2x), S1 = accum of both
        # lanes (DVE 4x) = P+Q, S2 = accum of lane0 (Act) = P.
        with tc.tile_pool(name="lay", bufs=1) as lay:
            lidx_sb = lay.tile([P, max(NIh[0], NIh[1]) // 16], i16)
            lmask_sb = lay.tile([P, Kmax * 32], bf16)
            nc.sync.dma_start(out=lmask_sb[:], in_=lmaskT)
            ltab = lay.tile([P, HALF_ROWS + 1], f32)
            nc.gpsimd.memset(ltab[:, HALF_ROWS:HALF_ROWS + 1], 0.0)
            S1h = lay.tile([P, 2, GPC], f32)
            S2h = lay.tile([P, 2, GPC], f32)

            for layer in (1, 2, 3):
                if layer == 1:
                    ang_src = ang1
                else:
                    ccab_in = dram2.tile([P, GPC], f32, tag="ccab_in")
                    ccab_out = dram2.tile([NC, P, GPC], f32, tag="ccab_out")
                    upkv = r3[:].bitcast(bf16).rearrange(
                        "p (g l) -> p g l", l=2)
                    nc.vector.tensor_mul(out=r1[:], in0=a_own[:],
                                         in1=dinv_sb[:])
                    nc.vector.tensor_mul(out=r2[:], in0=b_own[:],
                                         in1=dinv_sb[:])
                    nc.vector.tensor_copy(out=upkv[:, :, 0:1],
                                          in_=r1[:, :, None])
                    nc.vector.tensor_copy(out=upkv[:, :, 1:2],
                                          in_=r2[:, :, None])
                    nc.gpsimd.dma_start(out=ccab_in[:], in_=r3[:])
                    if not skip_cc:
                        nc.gpsimd.collective_compute(
                            "AllGather", mybir.AluOpType.bypass,
                            replica_groups=[list(range(NC))],
                            ins=[ccab_in.opt()], outs=[ccab_out.opt()])
                    else:
                        nc.gpsimd.dma_start(out=ccab_out[0], in_=ccab_in[:])
                    with tc.tile_pool(name=f"l{layer}", bufs=1) as lp, \
                         tc.tile_pool(name=f"l{layer}s", bufs=2) as lps:
                        for y in (0, 1):
                            half = ccab_out[4 * y:4 * y + 4].rearrange(
                                "r p g -> (r p g)")
                            nc.sync.dma_start(
                                out=ltab[:, 0:HALF_ROWS],
                                in_=half.partition_broadcast(P))
                            nc.sync.dma_start(
                                out=lidx_sb[:, 0:NIh[y] // 16], in_=lidxT[y])
                            gout = lp.tile([P, NIh[y], 1], f32, tag="gout")
                            nc.gpsimd.ap_gather(
                                out_ap=gout[:],
                                in_ap=ltab[:, :, None],
                                idxs_ap=lidx_sb[:, 0:NIh[y] // 16],
                                channels=P, num_elems=HALF_ROWS + 1, d=1,
                                num_idxs=NIh[y])
                            gb = gout[:, :, 0].bitcast(bf16)
                            for g in range(GPC):
                                kg = int(Kg[g, y])
                                o0 = int(LOFF[y][g]) * 32
                                mm = lps.tile([P, Kmax * 32], bf16, tag="mm")
                                nc.vector.tensor_mul(
                                    out=mm[:, 0:kg * 32],
                                    in0=gb[:, o0:o0 + kg * 32],
                                    in1=lmask_sb[:, 0:kg * 32])
                                nc.vector.tensor_scalar(
                                    out=mm[:, 0:kg * 32], in0=mm[:, 0:kg * 32],
                                    scalar1=1.0, scalar2=0.0, op0=OP.mult,
                                    op1=OP.add,
                                    accum_out=S1h[:, y, g:g + 1])
                                alane = mm[:, 0:kg * 32].rearrange(
                                    "p (s l) -> p s l", l=2)[:, :, 0]
                                nc.scalar.activation(
                                    alane, alane, AF.Copy,
                                    accum_out=S2h[:, y, g:g + 1])
                    nc.vector.tensor_add(out=P_all[:], in0=S2h[:, 0],
                                         in1=S2h[:, 1])
                    nc.vector.tensor_add(out=Q_all[:], in0=S1h[:, 0],
                                         in1=S1h[:, 1])
                    nc.vector.tensor_sub(out=Q_all[:], in0=Q_all[:],
                                         in1=P_all[:])
                    nc.vector.tensor_mul(out=r1[:], in0=P_all[:],
                                         in1=a_own[:])
                    nc.vector.tensor_mul(out=r2[:], in0=Q_all[:],
                                         in1=b_own[:])
                    nc.vector.tensor_add(out=r1[:], in0=r1[:], in1=r2[:])
                    nc.vector.tensor_mul(out=r1[:], in0=r1[:],
                                         in1=dinv_sb[:])
                    nc.vector.tensor_add(out=angL[:], in0=T_own[:],
                                         in1=r1[:])
                    ang_src = angL
                nc.scalar.activation(c_t[:], ang_src[:], AF.Sin,
                                     bias=halfpi[:])
                nc.scalar.activation(s_t[:], ang_src[:], AF.Sin)
                nc.vector.tensor_mul(out=r1[:], in0=c_t[:], in1=a_own[:])
                nc.vector.tensor_mul(out=r2[:], in0=s_t[:], in1=b_own[:])
                nc.vector.tensor_mul(out=r3[:], in0=s_t[:], in1=a_own[:])
                nc.vector.tensor_mul(out=r4[:], in0=c_t[:], in1=b_own[:])
                nc.vector.tensor_sub(out=a_own[:], in0=r1[:], in1=r2[:])
                nc.vector.tensor_add(out=b_own[:], in0=r3[:], in1=r4[:])

        # ---- classifier constants + write final (a,b)*d into hn_own ----
        cls = stack.enter_context(tc.tile_pool(name="cls", bufs=1))
        cw1b = cls.tile([P, 4, D], bf16)
        cw2b = cls.tile([P, 4, DOUT], bf16)
        ident = cls.tile([P, P], f32)
        nc.gpsimd.dma_start(out=cw1b[:], in_=cW1.rearrange(
            "(k p) f -> p k f", k=4, p=P))
        nc.gpsimd.dma_start(out=cw2b[:], in_=cW2.rearrange(
            "(k p) f -> p k f", k=4, p=P))
        make_identity(nc, ident[:])
        nc.vector.tensor_mul(out=r1[:], in0=a_own[:], in1=d_own[:])
        nc.vector.tensor_mul(out=r2[:], in0=b_own[:], in1=d_own[:])
        abw = cls.tile([P, GPC, 2], bf16)
        hTall = cls.tile([P, 4, NPC], bf16)
        nc.vector.tensor_copy(out=abw[:, :, 0:1], in_=r1[:, :, None])
        nc.vector.tensor_copy(out=abw[:, :, 1:2], in_=r2[:, :, None])
        nc.sync.dma_start(
            out=hn_own.rearrange("(g p) e -> p g e", g=GPC, p=P)[:, :, 0:2],
            in_=abw[:])
        for k in range(4):
            nc.sync.dma_start_transpose(out=hTall[:, k, :],
                                        in_=hn_own[:, k * P:(k + 1) * P])

        # ================= phase 5: classifier (transposed domain) ========
        with tc.tile_pool(name="p5", bufs=3) as p5, \
             tc.tile_pool(name="p5ps", bufs=2, space="PSUM") as p5ps:
            for g in range(GPC):
                zTd = p5ps.tile([P, 4, P], f32, tag="zT", space="PSUM")
                for of in range(4):
                    for k in range(4):
                        nc.tensor.matmul(out=zTd[:, of, :],
                                         lhsT=cw1b[:, k, of * P:(of + 1) * P],
                                         rhs=hTall[:, k, g * P:(g + 1) * P],
                                         start=(k == 0), stop=(k == 3))
                zr = p5.tile([P, 4, P], bf16, tag="zr")
                if not FL.get("cb1_zero", True):
                    for of in range(4):
                        nc.vector.tensor_scalar(
                            out=zr[:, of, :], in0=zTd[:, of, :],
                            scalar1=cb1mT[:, of:of + 1], scalar2=0.0,
                            op0=OP.add, op1=OP.max)
                else:
                    nc.vector.tensor_scalar_max(out=zr[:], in0=zTd[:], scalar1=0.0)
                z2 = p5.tile([P, 4, P], bf16, tag="z2")
                for of in range(4):
                    nc.vector.tensor_scalar(
                        out=z2[:, of, :], in0=zr[:, of, :],
                        scalar1=amT[:, of:of + 1], scalar2=bmT[:, of:of + 1],
                        op0=OP.mult, op1=OP.add)
                lgT = p5ps.tile([DOUT, P], f32, tag="lgps", space="PSUM")
                for k in range(4):
                    nc.tensor.matmul(out=lgT[:], lhsT=cw2b[:, k, :],
                                     rhs=z2[:, k, :],
                                     start=(k == 0), stop=(k == 3))
                lg_sb = p5.tile([DOUT, P], f32, tag="lgsb")
                nc.scalar.activation(lg_sb[:], lgT[:], AF.Copy)
                ptr = p5ps.tile([P, DOUT], f32, tag="tr", space="PSUM")
                nc.tensor.transpose(out=ptr[:], in_=lg_sb[:],
                                    identity=ident[0:DOUT, 0:DOUT])
                lgv = p5.tile([P, DOUT], f32, tag="lgv")
                if not FL.get("cb2_zero", True):
                    nc.vector.tensor_add(out=lgv[:], in0=ptr[:], in1=cb2m[:])
                else:
                    nc.vector.tensor_copy(out=lgv[:], in_=ptr[:])
                mx = p5.tile([P, 1], f32, tag="mx")
                nc.vector.reduce_max(out=mx[:], in_=lgv[:], axis=mybir.AxisListType.X)
                sh = p5.tile([P, DOUT], f32, tag="sh")
                nc.vector.tensor_scalar_sub(out=sh[:], in0=lgv[:], scalar1=mx[:])
                ex = p5.tile([P, DOUT], f32, tag="ex")
                se = p5.tile([P, 1], f32, tag="se")
                nc.scalar.activation(ex[:], sh[:], AF.Exp, accum_out=se[:])
                ls = p5.tile([P, 1], f32, tag="ls")
                nc.scalar.activation(ls[:], se[:], AF.Ln)
                ob = p5.tile([P, DOUT], f32, tag="ob")
                nc.vector.tensor_scalar_sub(out=ob[:], in0=sh[:], scalar1=ls[:])
                nc.sync.dma_start(out=out[g * P:(g + 1) * P, :], in_=ob[:])

    nc.compile()
    return nc


# ---------------------------------------------------------------- in_maps

def make_in_maps(cfg, percore, weights):
    ins = []
    for r in range(cfg.NC):
        pc = percore[r]
        m = dict(
            xT=pc["xT"],
            W_in=weights["W_in"], b_in=weights["b_in"][None, :],
            ln_g=weights["ln_g"][None, :], ln_b=weights["ln_b"][None, :],
            cW1=weights["cW1"], cb1=weights["cb1"][None, :],
            bn_g=weights["bn_g"][None, :], bn_b=weights["bn_b"][None, :],
            bn_m=weights["bn_mean"][None, :], bn_v=weights["bn_var"][None, :],
            cW2=weights["cW2"], cb2=weights["cb2"][None, :],
            srcl=pc["srcl"], dinv=pc["dinv"],
            yidx0=pc["yidx0"], yidx1=pc["yidx1"],
            lidx0=pc["lidx0"], lidx1=pc["lidx1"], lmask=pc["lmask"],
        )
        ins.append(m)
    return ins


def assemble_output(cfg, results, n):
    chunks = [results[r]["out"] for r in range(cfg.NC)]
    full = np.concatenate(chunks, axis=0)
    return full[:n]


# ---------------------------------------------------------------- entry point

def _cfg_flags(w):
    return dict(
        bin_zero=bool(np.all(w["b_in"] == 0)),
        ln_trivial=bool(np.all(w["ln_g"] == 1) and np.all(w["ln_b"] == 0)),
        cb1_zero=bool(np.all(w["cb1"] == 0)),
        cb2_zero=bool(np.all(w["cb2"] == 0)),
    )


def kernel(**inputs):
    """Full-input GNN forward on 8 TRN2 NeuronCores; returns [N, 40] fp32."""
    x = np.asarray(inputs["x"], np.float32)
    edge_src = np.asarray(inputs["edge_src"])
    edge_dst = np.asarray(inputs["edge_dst"])
    w = {k: np.asarray(inputs[k], np.float32) for k in
         ["W_in", "b_in", "ln_g", "ln_b", "cW1", "cb1", "bn_g", "bn_b",
          "bn_mean", "bn_var", "cW2", "cb2"]}
    N = x.shape[0]

    cfg, percore = host_prep(x, edge_src, edge_dst, n_cores=8)
    cfg.flags = _cfg_flags(w)
    nc = build_nc(cfg)
    in_maps = make_in_maps(cfg, percore, w)

    from concourse.bass_utils import run_bass_kernel_spmd
    res = run_bass_kernel_spmd(nc, in_maps, core_ids=list(range(cfg.NC)))
    return assemble_output(cfg, res.results, N).astype(np.float32)


def estimate_exec_ns(inputs):
    """Tile cost-model (TimelineSim) estimate of the per-core program span.

    Collective latencies are excluded (replaced by local shard copies to
    preserve the dependency structure); everything else is modeled."""
    x = np.asarray(inputs["x"], np.float32)
    w = {k: np.asarray(inputs[k], np.float32) for k in
         ["W_in", "b_in", "ln_g", "ln_b", "cW1", "cb1", "bn_g", "bn_b",
          "bn_mean", "bn_var", "cW2", "cb2"]}
    cfg, _ = host_prep(x, np.asarray(inputs["edge_src"]),
                       np.asarray(inputs["edge_dst"]), n_cores=8)
    cfg.flags = _cfg_flags(w)
    nc2 = build_nc(cfg, skip_cc=True)
    from concourse.timeline_sim import TimelineSim
    tl = TimelineSim(nc2, trace=False)
    ns = tl.simulate()
    return int(ns)



# revision 20
# speedup vs baseline: 1.3176x; 1.0159x over previous
"""GNN message-passing kernel for TRN2 (8 NeuronCores, SPMD) — v2.

Math (see reference):
  h = relu(x @ W_in + b_in);  h = LayerNorm(h) * ln_g + ln_b
  deg/dinv from edge_src;  hn = h / (||h|| + 1e-4)
  for 3 layers:
     ang_i = sum_{e: src=i} dinv_src*dinv_dst*<hn_src, hn_dst>
     rotate hn[:,0:2] by ang (Givens)
  z = relu(h @ cW1 + cb1); bn-affine; logits = z @ cW2 + cb2; log_softmax

Algebraic restructuring (as v1):
  - Givens preserves ||h||; only hn[:,0:2] changes across layers.
  - ang_i = <hn_i, M_i>, M_i = sum_e w_e * hn_dst  (w_e = dinv_src*dinv_dst)
  - T_i (tail, dims 2:512) constant across layers; per-layer head part
    uses fresh (a,b)=hn[:,0:2]:  ang_i = T_i + a_i*P_i + b_i*Q_i.

v2 distribution/layout changes vs v1:
  - Phase 0 computes ONLY own nodes (6272/core); the normalized features
    are AllGathered in fp8 (scaled by 8) instead of being recomputed
    8x redundantly on every core.
  - The AllGather OUTPUT BUFFER IS the gather table: node (r,g,p) lives
    at 512B row (r*128+p)*GPC+g.  The per-layer (a,b) AllGather uses the
    same row indexing with 256B rows ([GPC,64] f32 padded).  One set of
    int16 gather-index tables serves phase 3 and both layer gathers;
    class y = (dst core >= 4) splits rows into two halves for int16.
  - Phase-3 segment-sum matmuls run in fp8 DoubleRow (256-edge blocks).
  - Classifier matmuls run in f32r.
"""

import math
import numpy as np
import ml_dtypes

import sys as _sys
for _p in ("/opt/trn_rl_repo", "/root/.axon_site/_ro/trn_rl_repo"):
    if _p not in _sys.path:
        _sys.path.insert(0, _p)
import concourse.bacc as bacc
import concourse.tile as tile
import concourse.bass as bass
import concourse.mybir as mybir
from concourse.masks import make_identity

dt = mybir.dt
P = 128
D = 512
DOUT = 40
NC = 8
LN_EPS = 1e-5
BN_EPS = 1e-5
NRM_EPS = 1e-4
Y8 = 8.0          # fp8 feature prescale


class Cfg:
    def __init__(self, n_cores, gpc, B, flags, g3=2, vb=4, lkg=None):
        self.NC = n_cores
        self.GPC = gpc                   # groups (of 128 nodes) per core
        self.NPC = gpc * P               # nodes per core
        self.NPAD = n_cores * self.NPC
        self.ROWS = n_cores * P * gpc    # table rows (== NPAD)
        self.HALF_ROWS = self.ROWS // 2
        self.B = B                       # dict ycls -> blocks per group
        self.BT = B[0] + B[1]
        self.G3 = g3                     # phase-3 gather group batch
        self.VB = vb                     # phase-0 block batch
        self.LKG = lkg                   # per-(g, half) layer slots per node
        self.flags = flags


# ---------------------------------------------------------------- host prep

def host_prep(x, edge_src, edge_dst, n_cores=8, gpc=None):
    """Build per-core inputs + slot/index arrays. Returns (cfg, percore)."""
    N = x.shape[0]
    if gpc is None:
        gpc = (N + n_cores * P - 1) // (n_cores * P)
    NPC = gpc * P
    HALF_ROWS = n_cores * P * gpc // 2

    deg = np.bincount(edge_src, minlength=N).astype(np.float64)
    dinv = np.where(deg > 0, deg ** -0.5, 0.0).astype(np.float32)
    w_all = dinv[edge_src] * dinv[edge_dst]          # per-edge weight

    # table row of a node: (r*128 + p)*gpc + g
    def node_row(n):
        r = n // NPC
        nn = n % NPC
        g = nn // P
        p = nn % P
        return (r * P + p) * gpc + g

    src_core = edge_src // NPC
    percore_raw = []
    counts_all = np.zeros((n_cores, gpc, 2), np.int64)
    for r in range(n_cores):
        m = src_core == r
        es = edge_src[m]
        ww = w_all[m]
        rows = node_row(edge_dst[m].astype(np.int64))
        g = (es - r * NPC) // P
        ycls = (rows >= HALF_ROWS).astype(np.int64)
        key = (g * 2 + ycls).astype(np.int64)
        order = np.argsort(key, kind="stable")
        es, ww, rows, ycls = (a[order] for a in (es, ww, rows, ycls))
        counts_all[r] = np.bincount(key, minlength=gpc * 2).reshape(gpc, 2)
        percore_raw.append((es, ww, rows, ycls))

    kmax = counts_all.reshape(-1, 2).max(axis=0)
    # blocks per class: pad to 128 and round up to EVEN (DoubleRow pairs)
    B = {}
    for y in (0, 1):
        b = max(1, int((kmax[y] + P - 1) // P))
        B[y] = b + (b % 2)
    BT = B[0] + B[1]
    nslc = np.array([B[0] * P, B[1] * P], np.int64)
    slot_off = np.array([0, nslc[0]], np.int64)
    tot_slots = int(nslc.sum())

    xpad = np.zeros((n_cores * NPC, x.shape[1]), np.float32)
    xpad[:N] = x

    def wrap16(a2):      # [gpc, nslots] int16 -> [gpc, 128, nslots/16]
        w3 = a2.reshape(gpc, -1, 16).transpose(0, 2, 1)
        return np.ascontiguousarray(np.tile(w3, (1, 8, 1)))

    def slots_t(a2, s0, s1, nb):
        return a2[:, s0:s1].reshape(gpc, nb, P).transpose(0, 2, 1)

    percore = []
    for r in range(n_cores):
        es, ww, rows, ycls = percore_raw[r]
        cnt = counts_all[r]

        flat_starts = (np.arange(gpc)[:, None] * tot_slots + slot_off[None, :])
        csum = np.concatenate([[0], np.cumsum(cnt.reshape(-1))])[:-1].reshape(gpc, 2)
        e_idx = np.arange(len(es))
        bucket = ((es - r * NPC) // P) * 2 + ycls
        rank = e_idx - csum.reshape(-1)[bucket]
        slot = flat_starts.reshape(-1)[bucket] + rank

        srclf = np.full(gpc * tot_slots, -1.0, np.float32)
        yvf = np.zeros(gpc * tot_slots, np.int16)
        srclf[slot] = (es % P).astype(np.float32)
        yvf[slot] = (rows - ycls * HALF_ROWS).astype(np.int16)

        sf = srclf.reshape(gpc, tot_slots)
        yf = yvf.reshape(gpc, tot_slots)
        srcl = np.full((gpc, P, BT), -1.0, np.float32)
        yidx = {}
        boff = 0
        for y in (0, 1):
            s0, s1, nb = slot_off[y], slot_off[y] + nslc[y], B[y]
            srcl[:, :, boff:boff + nb] = slots_t(sf, s0, s1, nb)
            yidx[y] = wrap16(yf[:, s0:s1])
            boff += nb

        dinv_own = np.ascontiguousarray(
            dinv[np.arange(r * NPC, (r + 1) * NPC) % N].reshape(gpc, P)
            * (np.arange(r * NPC, (r + 1) * NPC) < N).reshape(gpc, P))
        xT_own = np.ascontiguousarray(xpad[r * NPC:(r + 1) * NPC].T)
        percore.append(dict(xT=xT_own,
                            srcl=srcl.astype(np.float32),
                            dinv=dinv_own.astype(np.float32),
                            yidx0=yidx[0], yidx1=yidx[1]))

    # ------- layer-gather tables (ap_gather from replicated half-tables) ---
    # Layers 2,3 fetch (a*dinv, b*dinv) per edge via GPSIMD ap_gather from a
    # [HALF_ROWS+1]-entry f32 half-table replicated across partitions (bf16
    # lanes pack the pair; entry HALF_ROWS is zero).  Per 16-partition group
    # G, half y, list position j = off(g)*16 + k*16 + pG holds the k-th
    # y-edge of node (16G+pG, g); k-major keeps the diagonal mask pattern
    # periodic in 32 elements independent of the per-(g, y) slot count K.
    Kg = np.zeros((gpc, 2), np.int64)
    pgk = []
    for r in range(n_cores):
        es, ww, rows, ycls = percore_raw[r]
        node = (es - r * NPC).astype(np.int64)
        key = node * 2 + ycls
        order = np.argsort(key, kind="stable")
        ks = np.empty(len(es), np.int64)
        csum = np.concatenate(
            [[0], np.cumsum(np.bincount(key, minlength=NPC * 2))])
        ks[order] = np.arange(len(es)) - csum[key[order]]
        cnt = np.bincount(key, minlength=NPC * 2).reshape(gpc, P, 2)
        Kg = np.maximum(Kg, cnt.max(axis=1))
        pgk.append((node % P, node // P, ks,
                    (rows - ycls * HALF_ROWS).astype(np.int64), ycls))
    Kg = Kg.astype(np.int64)
    offs = {y: np.concatenate([[0], np.cumsum(Kg[:, y])]) for y in (0, 1)}
    NIh = {y: 16 * int(offs[y][-1]) for y in (0, 1)}
    for r in range(n_cores):
        p, g, ks, loc, ycls = pgk[r]
        for y in (0, 1):
            m = ycls == y
            j = (offs[y][g[m]] + ks[m]) * 16 + (p[m] % 16)
            arr = np.full((P, NIh[y] // 16), HALF_ROWS, np.int16)
            arr[(p[m] // 16) * 16 + (j % 16), j // 16] = loc[m].astype(np.int16)
            percore[r][f"lidx{y}"] = np.ascontiguousarray(arr)
    Kmax = int(Kg.max())
    msk = np.zeros((P, Kmax, 16, 2), np.float32)
    for pp in range(P):
        msk[pp, :, pp % 16, :] = 1.0
    lmask = np.ascontiguousarray(
        msk.reshape(P, Kmax * 32)).astype(ml_dtypes.bfloat16)
    for r in range(n_cores):
        percore[r]["lmask"] = lmask

    cfg = Cfg(n_cores, gpc, B, {}, lkg=Kg)
    return cfg, percore


# ---------------------------------------------------------------- device build

def build_nc(cfg, skip_cc=False):
    GPC, NPC, ROWS, HALF_ROWS = cfg.GPC, cfg.NPC, cfg.ROWS, cfg.HALF_ROWS
    B, BT, G3, VB = cfg.B, cfg.BT, cfg.G3, cfg.VB
    Kg = cfg.LKG
    Kmax = int(Kg.max())
    LOFF = {y: np.concatenate([[0], np.cumsum(Kg[:, y])]).astype(int)
            for y in (0, 1)}
    NIh = {y: 16 * int(LOFF[y][-1]) for y in (0, 1)}
    FL = cfg.flags

    f32, f32r, bf16, i16 = dt.float32, dt.float32r, dt.bfloat16, dt.int16
    f8 = dt.float8e4
    AF = mybir.ActivationFunctionType
    OP = mybir.AluOpType
    DR = mybir.MatmulPerfMode.DoubleRow

    nc = bacc.Bacc("TRN2", target_bir_lowering=False, debug=False, num_devices=NC)

    # ---------------- I/O ----------------
    xT = nc.dram_tensor("xT", [D, NPC], f32, kind="ExternalInput").ap()
    W_in = nc.dram_tensor("W_in", [D, D], f32, kind="ExternalInput").ap()
    b_in = nc.dram_tensor("b_in", [1, D], f32, kind="ExternalInput").ap()
    ln_g = nc.dram_tensor("ln_g", [1, D], f32, kind="ExternalInput").ap()
    ln_b = nc.dram_tensor("ln_b", [1, D], f32, kind="ExternalInput").ap()
    cW1 = nc.dram_tensor("cW1", [D, D], f32, kind="ExternalInput").ap()
    cb1 = nc.dram_tensor("cb1", [1, D], f32, kind="ExternalInput").ap()
    bn_g = nc.dram_tensor("bn_g", [1, D], f32, kind="ExternalInput").ap()
    bn_b = nc.dram_tensor("bn_b", [1, D], f32, kind="ExternalInput").ap()
    bn_m = nc.dram_tensor("bn_m", [1, D], f32, kind="ExternalInput").ap()
    bn_v = nc.dram_tensor("bn_v", [1, D], f32, kind="ExternalInput").ap()
    cW2 = nc.dram_tensor("cW2", [D, DOUT], f32, kind="ExternalInput").ap()
    cb2 = nc.dram_tensor("cb2", [1, DOUT], f32, kind="ExternalInput").ap()
    srclT = nc.dram_tensor("srcl", [GPC, P, BT], f32, kind="ExternalInput").ap()
    dinvT = nc.dram_tensor("dinv", [GPC, P], f32, kind="ExternalInput").ap()
    yidxT = {}
    for y in (0, 1):
        yidxT[y] = nc.dram_tensor(f"yidx{y}", [GPC, P, B[y] * 8], i16,
                                  kind="ExternalInput").ap()
    lidxT = {}
    for y in (0, 1):
        lidxT[y] = nc.dram_tensor(f"lidx{y}", [P, NIh[y] // 16], i16,
                                  kind="ExternalInput").ap()
    lmaskT = nc.dram_tensor("lmask", [P, Kmax * 32], dt.bfloat16,
                            kind="ExternalInput").ap()
    out = nc.dram_tensor("out", [NPC, DOUT], f32, kind="ExternalOutput").ap()

    # ---------------- internal DRAM ----------------
    hn_own = nc.dram_tensor("hn_own", [NPC, D], bf16, kind="Internal").ap()

    from contextlib import ExitStack
    with tile.TileContext(nc) as tc, ExitStack() as stack:
        pers = stack.enter_context(tc.tile_pool(name="pers", bufs=1))
        dram = stack.enter_context(tc.tile_pool(name="dram", bufs=1, space="DRAM"))
        dram2 = stack.enter_context(tc.tile_pool(name="dram2", bufs=2, space="DRAM"))

        # collective buffers (DRAM); the OUT buffers are the gather tables
        ccy_in = dram.tile([P, GPC * D], f8, tag="ccy_in")
        ccy_out = dram.tile([NC, P, GPC * D], f8, tag="ccy_out")

        # persistent tiles
        halfpi = pers.tile([P, 1], f32)
        epsln = pers.tile([P, 1], f32)
        epsbn1 = pers.tile([1, 1], f32)
        cb1mT = pers.tile([P, 4], f32)
        dsc_t = pers.tile([P, GPC], f32)  # dinv/(8*(||h||+eps))
        a_own = pers.tile([P, GPC], f32)
        b_own = pers.tile([P, GPC], f32)
        d_own = pers.tile([P, GPC], f32)
        T_own = pers.tile([P, GPC], f32)
        ang1 = pers.tile([P, GPC], f32)
        P_all = pers.tile([P, GPC], f32)
        Q_all = pers.tile([P, GPC], f32)
        c_t = pers.tile([P, GPC], f32)
        s_t = pers.tile([P, GPC], f32)
        r1 = pers.tile([P, GPC], f32)
        r2 = pers.tile([P, GPC], f32)
        r3 = pers.tile([P, GPC], f32)
        r4 = pers.tile([P, GPC], f32)
        angL = pers.tile([P, GPC], f32)
        dinv_sb = pers.tile([P, GPC], f32)
        e8d = pers.tile([P, GPC], f32)       # 8 * dinv
        cb2m = pers.tile([P, DOUT], f32)

        # ---- one-time setup ----
        nc.sync.dma_start(out=dinv_sb[:], in_=dinvT.rearrange("g p -> p g"))
        nc.vector.tensor_scalar_mul(out=e8d[:], in0=dinv_sb[:], scalar1=Y8)
        nc.gpsimd.memset(halfpi[:], math.pi / 2)
        nc.gpsimd.memset(epsln[:], LN_EPS)
        nc.gpsimd.memset(epsbn1[:], BN_EPS)

        # (BN affine folded into cW2/cb2 on the host)
        if not FL.get("cb1_zero", True):
            nc.sync.dma_start(out=cb1mT[:], in_=cb1.rearrange("o (k p) -> p (o k)", k=4, p=P))
        if not FL.get("cb2_zero", True):
            bnt6 = pers.tile([1, DOUT], f32)
            nc.sync.dma_start(out=bnt6[:], in_=cb2[:])
            nc.gpsimd.partition_broadcast(cb2m[:], bnt6[:])
        iota_f = None  # created in the phase-3 scope

        # seq builder: one-hot(src%128) per slot, bf16 [P, BT, P].
        # One tensor_scalar per slot-block (2-dim APs keep the HW verifier
        # happy and hit the DVE fast path); w_e is factored out (dinv folded
        # into tables and per-partition scales).
        def build_seq(pool, g, iota_f, srcl_all, tag="selw"):
            seq = pool.tile([P, BT, P], bf16, tag=tag)
            for b in range(BT):
                nc.vector.tensor_scalar(
                    out=seq[:, b, :], in0=iota_f[:],
                    scalar1=srcl_all[:, g, b:b + 1], scalar2=None,
                    op0=OP.is_equal)
            return seq

        # ================= phase 0: dense + LN + normalize (own nodes) ====
        ccy_in_v = ccy_in[:].rearrange("p (g e) -> p g e", g=GPC, e=D)
        with tc.tile_pool(name="ph0c", bufs=1) as ph0c, \
             tc.tile_pool(name="p0", bufs=2) as p0, \
             tc.tile_pool(name="p0ps", bufs=3, space="PSUM") as p0ps:
            w_in_sb = ph0c.tile([P, 4, D], f32r)
            nc.sync.dma_start(out=w_in_sb[:], in_=W_in.rearrange(
                "(k p) f -> p k f", k=4, p=P).bitcast(f32r))
            gml = ph0c.tile([P, D], f32)
            bml = ph0c.tile([P, D], f32)
            binm = ph0c.tile([P, D], f32)
            bnt2 = ph0c.tile([1, D], f32)
            if not FL.get("ln_trivial", False):
                nc.sync.dma_start(out=bnt2[:], in_=ln_g[:])
                nc.gpsimd.partition_broadcast(gml[:], bnt2[:])
                nc.sync.dma_start(out=bnt2[:], in_=ln_b[:])
                nc.gpsimd.partition_broadcast(bml[:], bnt2[:])
            if not FL.get("bin_zero", True):
                nc.sync.dma_start(out=bnt2[:], in_=b_in[:])
                nc.gpsimd.partition_broadcast(binm[:], bnt2[:])
            n_batches = GPC // VB + (1 if GPC % VB else 0)
            for mb in range(n_batches):
                v0 = mb * VB
                nv = min(VB, GPC - v0)
                xb = p0.tile([P, VB, 4, P], f32r, tag="xb")
                xTr = xT.rearrange("(k p) (b n) -> p b k n", k=4, p=P, n=P)
                for v in range(nv):
                    nc.sync.dma_start(out=xb[:, v], in_=xTr[:, v0 + v].bitcast(f32r))
                hnb = p0.tile([P, VB, D], f32, tag="hnb")
                mu_s = p0.tile([P, VB], f32, tag="mu_s")
                var_s = p0.tile([P, VB], f32, tag="var_s")
                sd_t = p0.tile([P, VB], f32, tag="sd_t")
                istd = p0.tile([P, VB], f32, tag="istd")
                sv_t = p0.tile([P, VB], f32, tag="sv_t")
                nrm_t = p0.tile([P, VB], f32, tag="nrm_t")
                dba = p0.tile([P, VB], f32, tag="dba")
                idv = p0.tile([P, VB], f32, tag="idv")
                sc_t = p0.tile([P, VB], f32, tag="sc_t")
                hsb = []
                hcb = []
                for v in range(nv):
                    ph = p0ps.tile([P, D], f32, tag="ph", space="PSUM")
                    for k in range(4):
                        nc.tensor.matmul(out=ph[:], lhsT=xb[:, v, k, :],
                                         rhs=w_in_sb[:, k, :],
                                         start=(k == 0), stop=(k == 3))
                    h_sb = p0.tile([P, D], f32, tag=f"h{v}")
                    if not FL.get("bin_zero", True):
                        nc.vector.tensor_add(out=h_sb[:], in0=ph[:], in1=binm[:])
                        nc.vector.tensor_scalar_max(out=h_sb[:], in0=h_sb[:], scalar1=0.0)
                    else:
                        nc.vector.tensor_scalar_max(out=h_sb[:], in0=ph[:], scalar1=0.0)
                    nc.vector.reduce_sum(out=mu_s[:, v:v + 1], in_=h_sb[:],
                                         axis=mybir.AxisListType.X)
                    hsb.append(h_sb)
                nc.vector.tensor_scalar_mul(out=mu_s[:, 0:nv], in0=mu_s[:, 0:nv],
                                            scalar1=-1.0 / D)
                for v in range(nv):
                    hc = p0.tile([P, D], f32, tag=f"hc{v}")
                    nc.vector.tensor_scalar_add(out=hc[:], in0=hsb[v][:],
                                                scalar1=mu_s[:, v:v + 1])
                    sq = p0.tile([P, D], f32, tag="sq")
                    nc.scalar.activation(sq[:], hc[:], AF.Square,
                                         accum_out=var_s[:, v:v + 1])
                    hcb.append(hc)
                hf = p0.tile([P, VB, D], bf16, tag="hf")
                if FL.get("ln_trivial", True):
                    nc.scalar.activation(sd_t[:, 0:nv], var_s[:, 0:nv], AF.Sqrt,
                                         bias=epsln[:], scale=1.0 / D)
                    nc.vector.reciprocal(out=istd[:, 0:nv], in_=sd_t[:, 0:nv])
                    nc.scalar.activation(sv_t[:, 0:nv], var_s[:, 0:nv], AF.Sqrt)
                    nc.vector.tensor_mul(out=nrm_t[:, 0:nv], in0=istd[:, 0:nv],
                                         in1=sv_t[:, 0:nv])
                    nc.vector.tensor_scalar_add(out=dba[:, 0:nv], in0=nrm_t[:, 0:nv],
                                                scalar1=NRM_EPS)
                    nc.vector.reciprocal(out=idv[:, 0:nv], in_=dba[:, 0:nv])
                    nc.vector.tensor_mul(out=sc_t[:, 0:nv], in0=istd[:, 0:nv],
                                         in1=idv[:, 0:nv])
                    for v in range(nv):
                        nc.scalar.activation(hnb[:, v, :], hcb[v][:], AF.Copy,
                                             scale=sc_t[:, v:v + 1])
                        nc.scalar.activation(hf[:, v, :], hcb[v][:], AF.Copy,
                                             scale=istd[:, v:v + 1])
                    nc.vector.tensor_mul(out=dsc_t[:, v0:v0 + nv],
                                         in0=dinv_sb[:, v0:v0 + nv],
                                         in1=idv[:, 0:nv])
                else:
                    nc.scalar.activation(sd_t[:, 0:nv], var_s[:, 0:nv], AF.Sqrt,
                                         bias=epsln[:], scale=1.0 / D)
                    nc.vector.reciprocal(out=istd[:, 0:nv], in_=sd_t[:, 0:nv])
                    for v in range(nv):
                        hl = p0.tile([P, D], f32, tag=f"hl{v}")
                        nc.vector.scalar_tensor_tensor(
                            out=hl[:], in0=hcb[v][:], scalar=istd[:, v:v + 1],
                            in1=gml[:], op0=OP.mult, op1=OP.mult)
                        nc.vector.tensor_add(out=hl[:], in0=hl[:], in1=bml[:])
                        sq2 = p0.tile([P, D], f32, tag="sq")
                        nc.vector.scalar_tensor_tensor(
                            out=sq2[:], in0=hl[:], scalar=1.0, in1=hl[:],
                            op0=OP.mult, op1=OP.mult,
                            accum_out=nrm_t[:, v:v + 1])
                        hcb[v] = hl
                    nc.scalar.activation(sv_t[:, 0:nv], nrm_t[:, 0:nv], AF.Sqrt)
                    nc.vector.tensor_scalar_add(out=dba[:, 0:nv], in0=sv_t[:, 0:nv],
                                                scalar1=NRM_EPS)
                    nc.vector.reciprocal(out=sc_t[:, 0:nv], in_=dba[:, 0:nv])
                    for v in range(nv):
                        nc.scalar.activation(hnb[:, v, :], hcb[v][:], AF.Copy,
                                             scale=sc_t[:, v:v + 1])
                        nc.scalar.activation(hf[:, v, :], hcb[v][:], AF.Copy)
                    nc.vector.tensor_mul(out=dsc_t[:, v0:v0 + nv],
                                         in0=dinv_sb[:, v0:v0 + nv],
                                         in1=sc_t[:, 0:nv])
                # fp8 features (scaled by 8*dinv) -> collective input table
                y8 = p0.tile([P, VB, D], f8, tag="y8")
                for v in range(nv):
                    nc.vector.tensor_scalar_mul(
                        out=y8[:, v, :], in0=hnb[:, v, :],
                        scalar1=e8d[:, v0 + v:v0 + v + 1])
                nc.gpsimd.dma_start(out=ccy_in_v[:, v0:v0 + nv, :],
                                    in_=y8[:, 0:nv, :])
                nc.sync.dma_start(
                    out=hn_own[v0 * P:(v0 + nv) * P, :].rearrange(
                        "(v p) e -> p v e", v=nv, p=P),
                    in_=hf[:, 0:nv, :])
                for v in range(nv):
                    m = v0 + v
                    nc.vector.tensor_copy(out=a_own[:, m:m + 1], in_=hnb[:, v, 0:1])
                    nc.vector.tensor_copy(out=b_own[:, m:m + 1], in_=hnb[:, v, 1:2])
                    nc.vector.tensor_copy(out=d_own[:, m:m + 1], in_=dba[:, v:v + 1])

        nc.vector.tensor_scalar_mul(out=dsc_t[:], in0=dsc_t[:], scalar1=1.0 / Y8)

        # AllGather the fp8 feature table
        if not skip_cc:
            nc.gpsimd.collective_compute(
                "AllGather", mybir.AluOpType.bypass,
                replica_groups=[list(range(NC))],
                ins=[ccy_in.opt()], outs=[ccy_out.opt()])
        else:
            nc.gpsimd.dma_start(out=ccy_out[0], in_=ccy_in[:])

        yrows = ccy_out[:].rearrange("r p (g e) -> (r p g) e", g=GPC, e=D)

        # ================= phase 3: full-dot aggregation =================
        with tc.tile_pool(name="p3c", bufs=1) as p3c, \
             tc.tile_pool(name="p3", bufs=2) as p3, \
             tc.tile_pool(name="p3ps", bufs=2, space="PSUM") as p3ps:
            srcl_all = p3c.tile([P, GPC, BT], f32)
            nc.sync.dma_start(out=srcl_all[:],
                              in_=srclT.rearrange("g p s -> p g s"))
            iota_i = p3c.tile([P, P], dt.int32)
            iota_f = p3c.tile([P, P], bf16)
            nc.gpsimd.iota(iota_i[:], pattern=[[1, P]], base=0,
                           channel_multiplier=0)
            nc.vector.tensor_copy(out=iota_f[:], in_=iota_i[:])
            n3 = GPC // G3 + (1 if GPC % G3 else 0)
            for ib in range(n3):
                g0 = ib * G3
                gn = min(G3, GPC - g0)
                tg = {}
                for y in (0, 1):
                    s = B[y] * 8
                    tidx = p3.tile([P, G3 * s], i16, tag=f"yi{y}")
                    nc.sync.dma_start(
                        out=tidx[:, 0:gn * s].rearrange("p (g s) -> p g s",
                                                        g=gn, s=s),
                        in_=yidxT[y][g0:g0 + gn].rearrange("g p s -> p g s"))
                    t = p3.tile([P, G3 * B[y], D], f8, tag=f"tg{y}")
                    nc.gpsimd.dma_gather(
                        out_ap=t[:, 0:gn * B[y], :],
                        in_ap=yrows[y * HALF_ROWS:(y + 1) * HALF_ROWS, :],
                        idxs_ap=tidx[:, 0:gn * s],
                        num_idxs=gn * B[y] * P,
                        num_idxs_reg=gn * B[y] * P, elem_size=D,
                        single_packet=False)
                    tg[y] = t
                for gi in range(gn):
                    g = g0 + gi
                    seq = build_seq(p3, g, iota_f, srcl_all)
                    seq8 = p3.tile([P, BT, P], f8, tag="seq8")
                    nc.scalar.activation(seq8[:], seq[:], AF.Copy)
                    pm = p3ps.tile([P, D], f32, tag="M", space="PSUM")
                    nmm = BT // 2
                    mi = 0
                    for y in (0, 1):
                        for db in range(B[y] // 2):
                            boff = (0 if y == 0 else B[0]) + 2 * db
                            j0 = gi * B[y] + 2 * db
                            nc.tensor.matmul(
                                out=pm[:],
                                lhsT=seq8[:, boff:boff + 2, :],
                                rhs=tg[y][:, j0:j0 + 2, :],
                                start=(mi == 0), stop=(mi == nmm - 1),
                                perf_mode=DR)
                            mi += 1
                    hsl = p3.tile([P, D], bf16, tag="hsl")
                    nc.sync.dma_start(out=hsl[:], in_=hn_own[g * P:(g + 1) * P, :])
                    hs = p3.tile([P, D], f32, tag="hs")
                    nc.scalar.activation(hs[:], hsl[:], AF.Copy,
                                         scale=dsc_t[:, g:g + 1])
                    scr = p3.tile([P, D], f32, tag="scr")
                    nc.vector.tensor_mul(out=scr[:], in0=pm[:], in1=hs[:])
                    nc.vector.reduce_sum(out=ang1[:, g:g + 1], in_=scr[:],
                                         axis=mybir.AxisListType.X)
                    hsum = p3.tile([P, 1], f32, tag="hsum")
                    nc.vector.reduce_sum(out=hsum[:], in_=scr[:, 0:2],
                                         axis=mybir.AxisListType.X)
                    nc.vector.tensor_sub(out=T_own[:, g:g + 1],
                                         in0=ang1[:, g:g + 1], in1=hsum[:])

        # ================= layers =================
        # Layers 2,3: P_i = sum_e (a*dinv)_dst and Q_i likewise are fetched
        # per edge by ONE GPSIMD ap_gather per half from a partition-
        # replicated [HALF_ROWS+1]-entry f32 table (bf16 lanes pack the
        # pair; last entry is zero for padding slots).  k-major list order
        # makes the diagonal mask (pG == p%16) periodic in 32 elements, so
        # one [P, Kmax*32] mask serves every per-(g, half) slot count K.
        # Consume per (g, half): masked mul (DVE 2x), S1 = accum of both
        # lanes (DVE 4x) = P+Q, S2 = accum of lane0 (Act) = P.
        with tc.tile_pool(name="lay", bufs=1) as lay:
            lidx_sb = lay.tile([P, max(NIh[0], NIh[1]) // 16], i16)
            lmask_sb = lay.tile([P, Kmax * 32], bf16)
            nc.sync.dma_start(out=lmask_sb[:], in_=lmaskT)
            ltab = lay.tile([P, HALF_ROWS + 1], f32)
            nc.gpsimd.memset(ltab[:, HALF_ROWS:HALF_ROWS + 1], 0.0)
            S1h = lay.tile([P, 2, GPC], f32)
            S2h = lay.tile([P, 2, GPC], f32)

            for layer in (1, 2, 3):
                if layer == 1:
                    ang_src = ang1
                else:
                    ccab_in = dram2.tile([P, GPC], f32, tag="ccab_in")
                    ccab_out = dram2.tile([NC, P, GPC], f32, tag="ccab_out")
                    upkv = r3[:].bitcast(bf16).rearrange(
                        "p (g l) -> p g l", l=2)
                    nc.vector.tensor_mul(out=r1[:], in0=a_own[:],
                                         in1=dinv_sb[:])
                    nc.vector.tensor_mul(out=r2[:], in0=b_own[:],
                                         in1=dinv_sb[:])
                    nc.vector.tensor_copy(out=upkv[:, :, 0:1],
                                          in_=r1[:, :, None])
                    nc.vector.tensor_copy(out=upkv[:, :, 1:2],
                                          in_=r2[:, :, None])
                    nc.gpsimd.dma_start(out=ccab_in[:], in_=r3[:])
                    if not skip_cc:
                        nc.gpsimd.collective_compute(
                            "AllGather", mybir.AluOpType.bypass,
                            replica_groups=[list(range(NC))],
                            ins=[ccab_in.opt()], outs=[ccab_out.opt()])
                    else:
                        nc.gpsimd.dma_start(out=ccab_out[0], in_=ccab_in[:])
                    with tc.tile_pool(name=f"l{layer}", bufs=1) as lp, \
                         tc.tile_pool(name=f"l{layer}s", bufs=2) as lps:
                        for y in (0, 1):
                            half = ccab_out[4 * y:4 * y + 4].rearrange(
                                "r p g -> (r p g)")
                            nc.sync.dma_start(
                                out=ltab[:, 0:HALF_ROWS],
                                in_=half.partition_broadcast(P))
                            nc.sync.dma_start(
                                out=lidx_sb[:, 0:NIh[y] // 16], in_=lidxT[y])
                            gout = lp.tile([P, NIh[y], 1], f32, tag="gout")
                            nc.gpsimd.ap_gather(
                                out_ap=gout[:],
                                in_ap=ltab[:, :, None],
                                idxs_ap=lidx_sb[:, 0:NIh[y] // 16],
                                channels=P, num_elems=HALF_ROWS + 1, d=1,
                                num_idxs=NIh[y])
                            gb = gout[:, :, 0].bitcast(bf16)
                            for g in range(GPC):
                                kg = int(Kg[g, y])
                                o0 = int(LOFF[y][g]) * 32
                                mm = lps.tile([P, Kmax * 32], bf16, tag="mm")
                                nc.vector.tensor_mul(
                                    out=mm[:, 0:kg * 32],
                                    in0=gb[:, o0:o0 + kg * 32],
                                    in1=lmask_sb[:, 0:kg * 32])
                                nc.vector.tensor_scalar(
                                    out=mm[:, 0:kg * 32], in0=mm[:, 0:kg * 32],
                                    scalar1=1.0, scalar2=0.0, op0=OP.mult,
                                    op1=OP.add,
                                    accum_out=S1h[:, y, g:g + 1])
                                alane = mm[:, 0:kg * 32].rearrange(
                                    "p (s l) -> p s l", l=2)[:, :, 0]
                                nc.scalar.activation(
                                    alane, alane, AF.Copy,
                                    accum_out=S2h[:, y, g:g + 1])
                    nc.vector.tensor_add(out=P_all[:], in0=S2h[:, 0],
                                         in1=S2h[:, 1])
                    nc.vector.tensor_add(out=Q_all[:], in0=S1h[:, 0],
                                         in1=S1h[:, 1])
                    nc.vector.tensor_sub(out=Q_all[:], in0=Q_all[:],
                                         in1=P_all[:])
                    nc.vector.tensor_mul(out=r1[:], in0=P_all[:],
                                         in1=a_own[:])
                    nc.vector.tensor_mul(out=r2[:], in0=Q_all[:],
                                         in1=b_own[:])
                    nc.vector.tensor_add(out=r1[:], in0=r1[:], in1=r2[:])
                    nc.vector.tensor_mul(out=r1[:], in0=r1[:],
                                         in1=dinv_sb[:])
                    nc.vector.tensor_add(out=angL[:], in0=T_own[:],
                                         in1=r1[:])
                    ang_src = angL
                nc.scalar.activation(c_t[:], ang_src[:], AF.Sin,
                                     bias=halfpi[:])
                nc.scalar.activation(s_t[:], ang_src[:], AF.Sin)
                nc.vector.tensor_mul(out=r1[:], in0=c_t[:], in1=a_own[:])
                nc.vector.tensor_mul(out=r2[:], in0=s_t[:], in1=b_own[:])
                nc.vector.tensor_mul(out=r3[:], in0=s_t[:], in1=a_own[:])
                nc.vector.tensor_mul(out=r4[:], in0=c_t[:], in1=b_own[:])
                nc.vector.tensor_sub(out=a_own[:], in0=r1[:], in1=r2[:])
                nc.vector.tensor_add(out=b_own[:], in0=r3[:], in1=r4[:])

        # ---- classifier constants + write final (a,b)*d into hn_own ----
        cls = stack.enter_context(tc.tile_pool(name="cls", bufs=1))
        cw1b = cls.tile([P, 4, D], bf16)
        cw2b = cls.tile([P, 4, DOUT], bf16)
        ident = cls.tile([P, P], f32)
        nc.gpsimd.dma_start(out=cw1b[:], in_=cW1.rearrange(
            "(k p) f -> p k f", k=4, p=P))
        nc.gpsimd.dma_start(out=cw2b[:], in_=cW2.rearrange(
            "(k p) f -> p k f", k=4, p=P))
        make_identity(nc, ident[:])
        nc.vector.tensor_mul(out=r1[:], in0=a_own[:], in1=d_own[:])
        nc.vector.tensor_mul(out=r2[:], in0=b_own[:], in1=d_own[:])
        abw = cls.tile([P, GPC, 2], bf16)
        hTall = cls.tile([P, 4, NPC], bf16)
        nc.vector.tensor_copy(out=abw[:, :, 0:1], in_=r1[:, :, None])
        nc.vector.tensor_copy(out=abw[:, :, 1:2], in_=r2[:, :, None])
        nc.sync.dma_start(
            out=hn_own.rearrange("(g p) e -> p g e", g=GPC, p=P)[:, :, 0:2],
            in_=abw[:])
        for k in range(4):
            nc.sync.dma_start_transpose(out=hTall[:, k, :],
                                        in_=hn_own[:, k * P:(k + 1) * P])

        # ================= phase 5: classifier (transposed domain) ========
        # BN affine is folded into cW2/cb2 host-side; Exp stays resident on
        # the Act engine (log-sum Ln runs once, batched, at the end).
        with tc.tile_pool(name="p5", bufs=3) as p5, \
             tc.tile_pool(name="p5c", bufs=1) as p5c, \
             tc.tile_pool(name="p5ps", bufs=2, space="PSUM") as p5ps:
            sh_all = p5c.tile([P, GPC, DOUT], f32)
            se_all = p5c.tile([P, GPC], f32)
            ls_all = p5c.tile([P, GPC], f32)
            for g in range(GPC):
                zTd = p5ps.tile([P, 4, P], f32, tag="zT", space="PSUM")
                for of in range(4):
                    for k in range(4):
                        nc.tensor.matmul(out=zTd[:, of, :],
                                         lhsT=cw1b[:, k, of * P:(of + 1) * P],
                                         rhs=hTall[:, k, g * P:(g + 1) * P],
                                         start=(k == 0), stop=(k == 3))
                zr = p5.tile([P, 4, P], bf16, tag="zr")
                if not FL.get("cb1_zero", True):
                    for of in range(4):
                        nc.vector.tensor_scalar(
                            out=zr[:, of, :], in0=zTd[:, of, :],
                            scalar1=cb1mT[:, of:of + 1], scalar2=0.0,
                            op0=OP.add, op1=OP.max)
                else:
                    nc.vector.tensor_scalar_max(out=zr[:], in0=zTd[:], scalar1=0.0)
                lgT = p5ps.tile([DOUT, P], f32, tag="lgps", space="PSUM")
                for k in range(4):
                    nc.tensor.matmul(out=lgT[:], lhsT=cw2b[:, k, :],
                                     rhs=zr[:, k, :],
                                     start=(k == 0), stop=(k == 3))
                lg_sb = p5.tile([DOUT, P], f32, tag="lgsb")
                nc.vector.tensor_copy(out=lg_sb[:], in_=lgT[:])
                ptr = p5ps.tile([P, DOUT], f32, tag="tr", space="PSUM")
                nc.tensor.transpose(out=ptr[:], in_=lg_sb[:],
                                    identity=ident[0:DOUT, 0:DOUT])
                lgv = p5.tile([P, DOUT], f32, tag="lgv")
                if not FL.get("cb2_zero", True):
                    nc.vector.tensor_add(out=lgv[:], in0=ptr[:], in1=cb2m[:])
                else:
                    nc.vector.tensor_copy(out=lgv[:], in_=ptr[:])
                mx = p5.tile([P, 1], f32, tag="mx")
                nc.vector.reduce_max(out=mx[:], in_=lgv[:],
                                     axis=mybir.AxisListType.X)
                nc.vector.tensor_scalar_sub(out=sh_all[:, g, :], in0=lgv[:],
                                            scalar1=mx[:])
                ex = p5.tile([P, DOUT], f32, tag="ex")
                nc.scalar.activation(ex[:], sh_all[:, g, :], AF.Exp,
                                     accum_out=se_all[:, g:g + 1])
            nc.scalar.activation(ls_all[:], se_all[:], AF.Ln)
            for g in range(GPC):
                ob = p5.tile([P, DOUT], f32, tag="ob")
                nc.vector.tensor_scalar_sub(out=ob[:], in0=sh_all[:, g, :],
                                            scalar1=ls_all[:, g:g + 1])
                nc.sync.dma_start(out=out[g * P:(g + 1) * P, :], in_=ob[:])

    nc.compile()
    return nc


# ---------------------------------------------------------------- in_maps

def make_in_maps(cfg, percore, weights):
    ins = []
    for r in range(cfg.NC):
        pc = percore[r]
        m = dict(
            xT=pc["xT"],
            W_in=weights["W_in"], b_in=weights["b_in"][None, :],
            ln_g=weights["ln_g"][None, :], ln_b=weights["ln_b"][None, :],
            cW1=weights["cW1"], cb1=weights["cb1"][None, :],
            bn_g=weights["bn_g"][None, :], bn_b=weights["bn_b"][None, :],
            bn_m=weights["bn_mean"][None, :], bn_v=weights["bn_var"][None, :],
            cW2=weights["cW2"], cb2=weights["cb2"][None, :],
            srcl=pc["srcl"], dinv=pc["dinv"],
            yidx0=pc["yidx0"], yidx1=pc["yidx1"],
            lidx0=pc["lidx0"], lidx1=pc["lidx1"], lmask=pc["lmask"],
        )
        ins.append(m)
    return ins


def assemble_output(cfg, results, n):
    chunks = [results[r]["out"] for r in range(cfg.NC)]
    full = np.concatenate(chunks, axis=0)
    return full[:n]


# ---------------------------------------------------------------- entry point

def _fold_bn(w):
    """Fold the eval-mode BatchNorm affine into cW2 / cb2 (host-side)."""
    w = dict(w)
    alpha = (w["bn_g"] / np.sqrt(w["bn_var"] + BN_EPS)).astype(np.float64)
    beta = w["bn_b"] - w["bn_mean"] * alpha
    cb2 = beta.astype(np.float64) @ w["cW2"].astype(np.float64) + w["cb2"]
    w["cW2"] = (alpha[:, None] * w["cW2"]).astype(np.float32)
    w["cb2"] = cb2.astype(np.float32)
    return w


def _cfg_flags(w):
    return dict(
        bin_zero=bool(np.all(w["b_in"] == 0)),
        ln_trivial=bool(np.all(w["ln_g"] == 1) and np.all(w["ln_b"] == 0)),
        cb1_zero=bool(np.all(w["cb1"] == 0)),
        cb2_zero=bool(np.all(w["cb2"] == 0)),
    )


def kernel(**inputs):
    """Full-input GNN forward on 8 TRN2 NeuronCores; returns [N, 40] fp32."""
    x = np.asarray(inputs["x"], np.float32)
    edge_src = np.asarray(inputs["edge_src"])
    edge_dst = np.asarray(inputs["edge_dst"])
    w = {k: np.asarray(inputs[k], np.float32) for k in
         ["W_in", "b_in", "ln_g", "ln_b", "cW1", "cb1", "bn_g", "bn_b",
          "bn_mean", "bn_var", "cW2", "cb2"]}
    N = x.shape[0]

    w = _fold_bn(w)
    cfg, percore = host_prep(x, edge_src, edge_dst, n_cores=8)
    cfg.flags = _cfg_flags(w)
    nc = build_nc(cfg)
    in_maps = make_in_maps(cfg, percore, w)

    from concourse.bass_utils import run_bass_kernel_spmd
    res = run_bass_kernel_spmd(nc, in_maps, core_ids=list(range(cfg.NC)))
    return assemble_output(cfg, res.results, N).astype(np.float32)


def estimate_exec_ns(inputs):
    """Tile cost-model (TimelineSim) estimate of the per-core program span.

    Collective latencies are excluded (replaced by local shard copies to
    preserve the dependency structure); everything else is modeled."""
    x = np.asarray(inputs["x"], np.float32)
    w = {k: np.asarray(inputs[k], np.float32) for k in
         ["W_in", "b_in", "ln_g", "ln_b", "cW1", "cb1", "bn_g", "bn_b",
          "bn_mean", "bn_var", "cW2", "cb2"]}
    w = _fold_bn(w)
    cfg, _ = host_prep(x, np.asarray(inputs["edge_src"]),
                       np.asarray(inputs["edge_dst"]), n_cores=8)
    cfg.flags = _cfg_flags(w)
    nc2 = build_nc(cfg, skip_cc=True)
    from concourse.timeline_sim import TimelineSim
    tl = TimelineSim(nc2, trace=False)
    ns = tl.simulate()
    return int(ns)



# revision 23
# speedup vs baseline: 1.3595x; 1.0319x over previous
"""GNN message-passing kernel for TRN2 (8 NeuronCores, SPMD) — v2.

Math (see reference):
  h = relu(x @ W_in + b_in);  h = LayerNorm(h) * ln_g + ln_b
  deg/dinv from edge_src;  hn = h / (||h|| + 1e-4)
  for 3 layers:
     ang_i = sum_{e: src=i} dinv_src*dinv_dst*<hn_src, hn_dst>
     rotate hn[:,0:2] by ang (Givens)
  z = relu(h @ cW1 + cb1); bn-affine; logits = z @ cW2 + cb2; log_softmax

Algebraic restructuring (as v1):
  - Givens preserves ||h||; only hn[:,0:2] changes across layers.
  - ang_i = <hn_i, M_i>, M_i = sum_e w_e * hn_dst  (w_e = dinv_src*dinv_dst)
  - T_i (tail, dims 2:512) constant across layers; per-layer head part
    uses fresh (a,b)=hn[:,0:2]:  ang_i = T_i + a_i*P_i + b_i*Q_i.

v2 distribution/layout changes vs v1:
  - Phase 0 computes ONLY own nodes (6272/core); the normalized features
    are AllGathered in fp8 (scaled by 8) instead of being recomputed
    8x redundantly on every core.
  - The AllGather OUTPUT BUFFER IS the gather table: node (r,g,p) lives
    at 512B row (r*128+p)*GPC+g.  The per-layer (a,b) AllGather uses the
    same row indexing with 256B rows ([GPC,64] f32 padded).  One set of
    int16 gather-index tables serves phase 3 and both layer gathers;
    class y = (dst core >= 4) splits rows into two halves for int16.
  - Phase-3 segment-sum matmuls run in fp8 DoubleRow (256-edge blocks).
  - Classifier matmuls run in f32r.
"""

import math
import numpy as np
import ml_dtypes

import sys as _sys
for _p in ("/opt/trn_rl_repo", "/root/.axon_site/_ro/trn_rl_repo"):
    if _p not in _sys.path:
        _sys.path.insert(0, _p)
import concourse.bacc as bacc
import concourse.tile as tile
import concourse.bass as bass
import concourse.mybir as mybir
from concourse.masks import make_identity

dt = mybir.dt
P = 128
D = 512
DOUT = 40
NC = 8
LN_EPS = 1e-5
BN_EPS = 1e-5
NRM_EPS = 1e-4
Y8 = 8.0          # fp8 feature prescale


class Cfg:
    def __init__(self, n_cores, gpc, B, flags, g3=2, vb=4, lkg=None):
        self.NC = n_cores
        self.GPC = gpc                   # groups (of 128 nodes) per core
        self.NPC = gpc * P               # nodes per core
        self.NPAD = n_cores * self.NPC
        self.ROWS = n_cores * P * gpc    # table rows (== NPAD)
        self.HALF_ROWS = self.ROWS // 2
        self.B = B                       # dict ycls -> blocks per group
        self.BT = B[0] + B[1]
        self.G3 = g3                     # phase-3 gather group batch
        self.VB = vb                     # phase-0 block batch
        self.LKG = lkg                   # per-(g, half) layer slots per node
        self.flags = flags


# ---------------------------------------------------------------- host prep

def host_prep(x, edge_src, edge_dst, n_cores=8, gpc=None):
    """Build per-core inputs + slot/index arrays. Returns (cfg, percore)."""
    N = x.shape[0]
    if gpc is None:
        gpc = (N + n_cores * P - 1) // (n_cores * P)
    NPC = gpc * P
    HALF_ROWS = n_cores * P * gpc // 2

    deg = np.bincount(edge_src, minlength=N).astype(np.float64)
    dinv = np.where(deg > 0, deg ** -0.5, 0.0).astype(np.float32)
    w_all = dinv[edge_src] * dinv[edge_dst]          # per-edge weight

    # table row of a node: (r*128 + p)*gpc + g
    def node_row(n):
        r = n // NPC
        nn = n % NPC
        g = nn // P
        p = nn % P
        return (r * P + p) * gpc + g

    src_core = edge_src // NPC
    percore_raw = []
    counts_all = np.zeros((n_cores, gpc, 2), np.int64)
    for r in range(n_cores):
        m = src_core == r
        es = edge_src[m]
        ww = w_all[m]
        rows = node_row(edge_dst[m].astype(np.int64))
        g = (es - r * NPC) // P
        ycls = (rows >= HALF_ROWS).astype(np.int64)
        key = (g * 2 + ycls).astype(np.int64)
        order = np.argsort(key, kind="stable")
        es, ww, rows, ycls = (a[order] for a in (es, ww, rows, ycls))
        counts_all[r] = np.bincount(key, minlength=gpc * 2).reshape(gpc, 2)
        percore_raw.append((es, ww, rows, ycls))

    kmax = counts_all.reshape(-1, 2).max(axis=0)
    # blocks per class: pad to 128 and round up to EVEN (DoubleRow pairs)
    B = {}
    for y in (0, 1):
        b = max(1, int((kmax[y] + P - 1) // P))
        B[y] = b + (b % 2)
    BT = B[0] + B[1]
    nslc = np.array([B[0] * P, B[1] * P], np.int64)
    slot_off = np.array([0, nslc[0]], np.int64)
    tot_slots = int(nslc.sum())

    xpad = np.zeros((n_cores * NPC, x.shape[1]), np.float32)
    xpad[:N] = x

    def wrap16(a2):      # [gpc, nslots] int16 -> [gpc, 128, nslots/16]
        w3 = a2.reshape(gpc, -1, 16).transpose(0, 2, 1)
        return np.ascontiguousarray(np.tile(w3, (1, 8, 1)))

    def slots_t(a2, s0, s1, nb):
        return a2[:, s0:s1].reshape(gpc, nb, P).transpose(0, 2, 1)

    percore = []
    for r in range(n_cores):
        es, ww, rows, ycls = percore_raw[r]
        cnt = counts_all[r]

        flat_starts = (np.arange(gpc)[:, None] * tot_slots + slot_off[None, :])
        csum = np.concatenate([[0], np.cumsum(cnt.reshape(-1))])[:-1].reshape(gpc, 2)
        e_idx = np.arange(len(es))
        bucket = ((es - r * NPC) // P) * 2 + ycls
        rank = e_idx - csum.reshape(-1)[bucket]
        slot = flat_starts.reshape(-1)[bucket] + rank

        srclf = np.full(gpc * tot_slots, -1.0, np.float32)
        yvf = np.zeros(gpc * tot_slots, np.int16)
        srclf[slot] = (es % P).astype(np.float32)
        yvf[slot] = (rows - ycls * HALF_ROWS).astype(np.int16)

        sf = srclf.reshape(gpc, tot_slots)
        yf = yvf.reshape(gpc, tot_slots)
        srcl = np.full((gpc, P, BT), -1.0, np.float32)
        yidx = {}
        boff = 0
        for y in (0, 1):
            s0, s1, nb = slot_off[y], slot_off[y] + nslc[y], B[y]
            srcl[:, :, boff:boff + nb] = slots_t(sf, s0, s1, nb)
            yidx[y] = wrap16(yf[:, s0:s1])
            boff += nb

        dinv_own = np.ascontiguousarray(
            dinv[np.arange(r * NPC, (r + 1) * NPC) % N].reshape(gpc, P)
            * (np.arange(r * NPC, (r + 1) * NPC) < N).reshape(gpc, P))
        xT_own = np.ascontiguousarray(xpad[r * NPC:(r + 1) * NPC].T)
        percore.append(dict(xT=xT_own,
                            srcl=srcl.astype(np.float32),
                            dinv=dinv_own.astype(np.float32),
                            yidx0=yidx[0], yidx1=yidx[1]))

    # ------- layer-gather tables (ap_gather from replicated half-tables) ---
    # Layers 2,3 fetch (a*dinv, b*dinv) per edge via GPSIMD ap_gather from a
    # [HALF_ROWS+1]-entry f32 half-table replicated across partitions (bf16
    # lanes pack the pair; entry HALF_ROWS is zero).  Per 16-partition group
    # G, half y, list position j = off(g)*16 + k*16 + pG holds the k-th
    # y-edge of node (16G+pG, g); k-major keeps the diagonal mask pattern
    # periodic in 32 elements independent of the per-(g, y) slot count K.
    Kg = np.zeros((gpc, 2), np.int64)
    pgk = []
    for r in range(n_cores):
        es, ww, rows, ycls = percore_raw[r]
        node = (es - r * NPC).astype(np.int64)
        key = node * 2 + ycls
        order = np.argsort(key, kind="stable")
        ks = np.empty(len(es), np.int64)
        csum = np.concatenate(
            [[0], np.cumsum(np.bincount(key, minlength=NPC * 2))])
        ks[order] = np.arange(len(es)) - csum[key[order]]
        cnt = np.bincount(key, minlength=NPC * 2).reshape(gpc, P, 2)
        Kg = np.maximum(Kg, cnt.max(axis=1))
        pgk.append((node % P, node // P, ks,
                    (rows - ycls * HALF_ROWS).astype(np.int64), ycls))
    Kg = Kg.astype(np.int64)
    offs = {y: np.concatenate([[0], np.cumsum(Kg[:, y])]) for y in (0, 1)}
    NIh = {y: 16 * int(offs[y][-1]) for y in (0, 1)}
    for r in range(n_cores):
        p, g, ks, loc, ycls = pgk[r]
        for y in (0, 1):
            m = ycls == y
            j = (offs[y][g[m]] + ks[m]) * 16 + (p[m] % 16)
            arr = np.full((P, NIh[y] // 16), HALF_ROWS, np.int16)
            arr[(p[m] // 16) * 16 + (j % 16), j // 16] = loc[m].astype(np.int16)
            percore[r][f"lidx{y}"] = np.ascontiguousarray(arr)
    Kmax = int(Kg.max())
    msk = np.zeros((P, Kmax, 16, 2), np.float32)
    for pp in range(P):
        msk[pp, :, pp % 16, :] = 1.0
    lmask = np.ascontiguousarray(
        msk.reshape(P, Kmax * 32)).astype(ml_dtypes.bfloat16)
    for r in range(n_cores):
        percore[r]["lmask"] = lmask

    cfg = Cfg(n_cores, gpc, B, {}, lkg=Kg)
    return cfg, percore


# ---------------------------------------------------------------- device build

def build_nc(cfg, skip_cc=False):
    GPC, NPC, ROWS, HALF_ROWS = cfg.GPC, cfg.NPC, cfg.ROWS, cfg.HALF_ROWS
    B, BT, G3, VB = cfg.B, cfg.BT, cfg.G3, cfg.VB
    Kg = cfg.LKG
    Kmax = int(Kg.max())
    LOFF = {y: np.concatenate([[0], np.cumsum(Kg[:, y])]).astype(int)
            for y in (0, 1)}
    NIh = {y: 16 * int(LOFF[y][-1]) for y in (0, 1)}
    FL = cfg.flags

    f32, f32r, bf16, i16 = dt.float32, dt.float32r, dt.bfloat16, dt.int16
    f8 = dt.float8e4
    AF = mybir.ActivationFunctionType
    OP = mybir.AluOpType
    DR = mybir.MatmulPerfMode.DoubleRow

    nc = bacc.Bacc("TRN2", target_bir_lowering=False, debug=False, num_devices=NC)

    # ---------------- I/O ----------------
    xT = nc.dram_tensor("xT", [D, NPC], f32, kind="ExternalInput").ap()
    W_in = nc.dram_tensor("W_in", [D, D], f32, kind="ExternalInput").ap()
    b_in = nc.dram_tensor("b_in", [1, D], f32, kind="ExternalInput").ap()
    ln_g = nc.dram_tensor("ln_g", [1, D], f32, kind="ExternalInput").ap()
    ln_b = nc.dram_tensor("ln_b", [1, D], f32, kind="ExternalInput").ap()
    cW1 = nc.dram_tensor("cW1", [D, D], f32, kind="ExternalInput").ap()
    cb1 = nc.dram_tensor("cb1", [1, D], f32, kind="ExternalInput").ap()
    bn_g = nc.dram_tensor("bn_g", [1, D], f32, kind="ExternalInput").ap()
    bn_b = nc.dram_tensor("bn_b", [1, D], f32, kind="ExternalInput").ap()
    bn_m = nc.dram_tensor("bn_m", [1, D], f32, kind="ExternalInput").ap()
    bn_v = nc.dram_tensor("bn_v", [1, D], f32, kind="ExternalInput").ap()
    cW2 = nc.dram_tensor("cW2", [D, DOUT], f32, kind="ExternalInput").ap()
    cb2 = nc.dram_tensor("cb2", [1, DOUT], f32, kind="ExternalInput").ap()
    srclT = nc.dram_tensor("srcl", [GPC, P, BT], f32, kind="ExternalInput").ap()
    dinvT = nc.dram_tensor("dinv", [GPC, P], f32, kind="ExternalInput").ap()
    yidxT = {}
    for y in (0, 1):
        yidxT[y] = nc.dram_tensor(f"yidx{y}", [GPC, P, B[y] * 8], i16,
                                  kind="ExternalInput").ap()
    lidxT = {}
    for y in (0, 1):
        lidxT[y] = nc.dram_tensor(f"lidx{y}", [P, NIh[y] // 16], i16,
                                  kind="ExternalInput").ap()
    lmaskT = nc.dram_tensor("lmask", [P, Kmax * 32], dt.bfloat16,
                            kind="ExternalInput").ap()
    out = nc.dram_tensor("out", [NPC, DOUT], f32, kind="ExternalOutput").ap()

    # ---------------- internal DRAM ----------------
    hn_own = nc.dram_tensor("hn_own", [NPC, D], bf16, kind="Internal").ap()

    from contextlib import ExitStack
    with tile.TileContext(nc) as tc, ExitStack() as stack:
        pers = stack.enter_context(tc.tile_pool(name="pers", bufs=1))
        dram = stack.enter_context(tc.tile_pool(name="dram", bufs=1, space="DRAM"))
        dram2 = stack.enter_context(tc.tile_pool(name="dram2", bufs=2, space="DRAM"))

        # collective buffers (DRAM); the OUT buffers are the gather tables
        ccy_in = dram.tile([P, GPC * D], f8, tag="ccy_in")
        ccy_out = dram.tile([NC, P, GPC * D], f8, tag="ccy_out")

        # persistent tiles
        halfpi = pers.tile([P, 1], f32)
        epsln = pers.tile([P, 1], f32)
        epsbn1 = pers.tile([1, 1], f32)
        cb1mT = pers.tile([P, 4], f32)
        dsc_t = pers.tile([P, GPC], f32)  # dinv/(8*(||h||+eps))
        a_own = pers.tile([P, GPC], f32)
        b_own = pers.tile([P, GPC], f32)
        d_own = pers.tile([P, GPC], f32)
        T_own = pers.tile([P, GPC], f32)
        ang1 = pers.tile([P, GPC], f32)
        P_all = pers.tile([P, GPC], f32)
        Q_all = pers.tile([P, GPC], f32)
        c_t = pers.tile([P, GPC], f32)
        s_t = pers.tile([P, GPC], f32)
        r1 = pers.tile([P, GPC], f32)
        r2 = pers.tile([P, GPC], f32)
        r3 = pers.tile([P, GPC], f32)
        r4 = pers.tile([P, GPC], f32)
        angL = pers.tile([P, GPC], f32)
        dinv_sb = pers.tile([P, GPC], f32)
        e8d = pers.tile([P, GPC], f32)       # 8 * dinv
        cb2m = pers.tile([P, DOUT], f32)

        # ---- one-time setup ----
        nc.sync.dma_start(out=dinv_sb[:], in_=dinvT.rearrange("g p -> p g"))
        nc.vector.tensor_scalar_mul(out=e8d[:], in0=dinv_sb[:], scalar1=Y8)
        nc.gpsimd.memset(halfpi[:], math.pi / 2)
        nc.gpsimd.memset(epsln[:], LN_EPS)
        nc.gpsimd.memset(epsbn1[:], BN_EPS)

        # (BN affine folded into cW2/cb2 on the host)
        if not FL.get("cb1_zero", True):
            nc.sync.dma_start(out=cb1mT[:], in_=cb1.rearrange("o (k p) -> p (o k)", k=4, p=P))
        if not FL.get("cb2_zero", True):
            bnt6 = pers.tile([1, DOUT], f32)
            nc.sync.dma_start(out=bnt6[:], in_=cb2[:])
            nc.gpsimd.partition_broadcast(cb2m[:], bnt6[:])
        iota_f = None  # created in the phase-3 scope

        # seq builder: one-hot(src%128) per slot, bf16 [P, BT, P].
        # One tensor_scalar per slot-block (2-dim APs keep the HW verifier
        # happy and hit the DVE fast path); w_e is factored out (dinv folded
        # into tables and per-partition scales).
        def build_seq(pool, g, iota_f, srcl_all, tag="selw"):
            seq = pool.tile([P, BT, P], bf16, tag=tag)
            for b in range(BT):
                nc.vector.tensor_scalar(
                    out=seq[:, b, :], in0=iota_f[:],
                    scalar1=srcl_all[:, g, b:b + 1], scalar2=None,
                    op0=OP.is_equal)
            return seq

        # ================= phase 0: dense + LN + normalize (own nodes) ====
        ccy_in_v = ccy_in[:].rearrange("p (g e) -> p g e", g=GPC, e=D)
        with tc.tile_pool(name="ph0c", bufs=1) as ph0c, \
             tc.tile_pool(name="p0", bufs=2) as p0, \
             tc.tile_pool(name="p0ps", bufs=3, space="PSUM") as p0ps:
            w_in_sb = ph0c.tile([P, 4, D], f32r)
            nc.sync.dma_start(out=w_in_sb[:], in_=W_in.rearrange(
                "(k p) f -> p k f", k=4, p=P).bitcast(f32r))
            gml = ph0c.tile([P, D], f32)
            bml = ph0c.tile([P, D], f32)
            binm = ph0c.tile([P, D], f32)
            bnt2 = ph0c.tile([1, D], f32)
            if not FL.get("ln_trivial", False):
                nc.sync.dma_start(out=bnt2[:], in_=ln_g[:])
                nc.gpsimd.partition_broadcast(gml[:], bnt2[:])
                nc.sync.dma_start(out=bnt2[:], in_=ln_b[:])
                nc.gpsimd.partition_broadcast(bml[:], bnt2[:])
            if not FL.get("bin_zero", True):
                nc.sync.dma_start(out=bnt2[:], in_=b_in[:])
                nc.gpsimd.partition_broadcast(binm[:], bnt2[:])
            n_batches = GPC // VB + (1 if GPC % VB else 0)
            for mb in range(n_batches):
                v0 = mb * VB
                nv = min(VB, GPC - v0)
                xb = p0.tile([P, VB, 4, P], f32r, tag="xb")
                xTr = xT.rearrange("(k p) (b n) -> p b k n", k=4, p=P, n=P)
                for v in range(nv):
                    nc.sync.dma_start(out=xb[:, v], in_=xTr[:, v0 + v].bitcast(f32r))
                hsum = p0.tile([P, VB], f32, tag="hsum")
                ss = p0.tile([P, VB], f32, tag="ss")
                mu = p0.tile([P, VB], f32, tag="mu")      # holds -mean
                ek = p0.tile([P, VB], f32, tag="ek")
                var_s = p0.tile([P, VB], f32, tag="var_s")
                istd = p0.tile([P, VB], f32, tag="istd")
                sv_t = p0.tile([P, VB], f32, tag="sv_t")
                dba = p0.tile([P, VB], f32, tag="dba")
                idv = p0.tile([P, VB], f32, tag="idv")
                sc_t = p0.tile([P, VB], f32, tag="sc_t")
                y8s = p0.tile([P, VB], f32, tag="y8s")
                y8b = p0.tile([P, VB], f32, tag="y8b")
                hfb = p0.tile([P, VB], f32, tag="hfb")
                absc = p0.tile([P, VB], f32, tag="absc")
                hsb = []
                # pass 1: matmul, relu (+row-sum), square (+row-sum)
                for v in range(nv):
                    ph = p0ps.tile([P, D], f32, tag="ph", space="PSUM")
                    for k in range(4):
                        nc.tensor.matmul(out=ph[:], lhsT=xb[:, v, k, :],
                                         rhs=w_in_sb[:, k, :],
                                         start=(k == 0), stop=(k == 3))
                    h_sb = p0.tile([P, D], bf16, tag=f"h{v}")
                    if not FL.get("bin_zero", True):
                        hb = p0.tile([P, D], f32, tag="hb")
                        nc.vector.tensor_add(out=hb[:], in0=ph[:], in1=binm[:])
                        nc.vector.tensor_scalar(
                            out=h_sb[:], in0=hb[:], scalar1=0.0, scalar2=0.0,
                            op0=OP.max, op1=OP.add,
                            accum_out=hsum[:, v:v + 1])
                    else:
                        nc.vector.tensor_scalar(
                            out=h_sb[:], in0=ph[:], scalar1=0.0, scalar2=0.0,
                            op0=OP.max, op1=OP.add,
                            accum_out=hsum[:, v:v + 1])
                    sqj = p0.tile([P, D], bf16, tag="sqj")
                    nc.scalar.activation(sqj[:], h_sb[:], AF.Square,
                                         accum_out=ss[:, v:v + 1])
                    hsb.append(h_sb)
                # batched LN stats: var = E[h^2] - mean^2 (mu holds -mean)
                nc.vector.tensor_scalar_mul(out=mu[:, 0:nv], in0=hsum[:, 0:nv],
                                            scalar1=-1.0 / D)
                nc.vector.tensor_scalar_mul(out=ek[:, 0:nv], in0=ss[:, 0:nv],
                                            scalar1=1.0 / D)
                nc.vector.tensor_mul(out=var_s[:, 0:nv], in0=mu[:, 0:nv],
                                     in1=mu[:, 0:nv])
                nc.vector.tensor_sub(out=var_s[:, 0:nv], in0=ek[:, 0:nv],
                                     in1=var_s[:, 0:nv])
                sd_t = p0.tile([P, VB], f32, tag="sd_t")
                nc.scalar.activation(sd_t[:, 0:nv], var_s[:, 0:nv], AF.Sqrt,
                                     bias=epsln[:])
                nc.vector.reciprocal(out=istd[:, 0:nv], in_=sd_t[:, 0:nv])
                hf = p0.tile([P, VB, D], bf16, tag="hf")
                y8 = p0.tile([P, VB, D], f8, tag="y8")
                if FL.get("ln_trivial", True):
                    # ||LN(h)|| = sqrt(D*var) * istd
                    nc.scalar.activation(sv_t[:, 0:nv], var_s[:, 0:nv],
                                         AF.Sqrt, scale=float(D))
                    nc.vector.tensor_mul(out=dba[:, 0:nv], in0=sv_t[:, 0:nv],
                                         in1=istd[:, 0:nv])
                    nc.vector.tensor_scalar_add(out=dba[:, 0:nv],
                                                in0=dba[:, 0:nv],
                                                scalar1=NRM_EPS)
                    nc.vector.reciprocal(out=idv[:, 0:nv], in_=dba[:, 0:nv])
                    nc.vector.tensor_mul(out=sc_t[:, 0:nv], in0=istd[:, 0:nv],
                                         in1=idv[:, 0:nv])
                    nc.vector.tensor_mul(out=y8s[:, 0:nv], in0=sc_t[:, 0:nv],
                                         in1=e8d[:, v0:v0 + nv])
                    nc.vector.tensor_mul(out=dsc_t[:, v0:v0 + nv],
                                         in0=dinv_sb[:, v0:v0 + nv],
                                         in1=idv[:, 0:nv])
                    # pass 2: center (DVE 4x), fp8 rows (Act), bf16 h (DVE)
                    for v in range(nv):
                        hcv = p0.tile([P, D], bf16, tag=f"hc{v}")
                        nc.vector.tensor_scalar_add(out=hcv[:], in0=hsb[v][:],
                                                    scalar1=mu[:, v:v + 1])
                        nc.scalar.activation(y8[:, v, :], hcv[:], AF.Copy,
                                             scale=y8s[:, v:v + 1])
                        nc.vector.tensor_scalar_mul(
                            out=hf[:, v, :], in0=hcv[:],
                            scalar1=istd[:, v:v + 1])
                        ab2 = p0.tile([P, 2], f32, tag="ab2")
                        nc.vector.tensor_scalar_mul(
                            out=ab2[:], in0=hcv[:, 0:2],
                            scalar1=sc_t[:, v:v + 1])
                        m = v0 + v
                        nc.vector.tensor_copy(out=a_own[:, m:m + 1],
                                              in_=ab2[:, 0:1])
                        nc.vector.tensor_copy(out=b_own[:, m:m + 1],
                                              in_=ab2[:, 1:2])
                        nc.vector.tensor_copy(out=d_own[:, m:m + 1],
                                              in_=dba[:, v:v + 1])
                else:
                    # general LN path: hl = (h - mean)*istd*ln_g + ln_b
                    nrm_t = p0.tile([P, VB], f32, tag="nrm_t")
                    for v in range(nv):
                        hc = p0.tile([P, D], f32, tag="hc")
                        nc.vector.tensor_scalar_add(out=hc[:], in0=hsb[v][:],
                                                    scalar1=mu[:, v:v + 1])
                        hl = p0.tile([P, D], f32, tag=f"hl{v}")
                        nc.vector.scalar_tensor_tensor(
                            out=hl[:], in0=hc[:], scalar=istd[:, v:v + 1],
                            in1=gml[:], op0=OP.mult, op1=OP.mult)
                        nc.vector.tensor_add(out=hl[:], in0=hl[:], in1=bml[:])
                        sq2 = p0.tile([P, D], f32, tag="sq2")
                        nc.vector.scalar_tensor_tensor(
                            out=sq2[:], in0=hl[:], scalar=1.0, in1=hl[:],
                            op0=OP.mult, op1=OP.mult,
                            accum_out=nrm_t[:, v:v + 1])
                        hsb[v] = hl
                    nc.scalar.activation(sv_t[:, 0:nv], nrm_t[:, 0:nv],
                                         AF.Sqrt)
                    nc.vector.tensor_scalar_add(out=dba[:, 0:nv],
                                                in0=sv_t[:, 0:nv],
                                                scalar1=NRM_EPS)
                    nc.vector.reciprocal(out=sc_t[:, 0:nv], in_=dba[:, 0:nv])
                    nc.vector.tensor_mul(out=y8s[:, 0:nv], in0=sc_t[:, 0:nv],
                                         in1=e8d[:, v0:v0 + nv])
                    nc.vector.tensor_mul(out=dsc_t[:, v0:v0 + nv],
                                         in0=dinv_sb[:, v0:v0 + nv],
                                         in1=sc_t[:, 0:nv])
                    for v in range(nv):
                        nc.scalar.activation(y8[:, v, :], hsb[v][:], AF.Copy,
                                             scale=y8s[:, v:v + 1])
                        nc.vector.tensor_copy(out=hf[:, v, :], in_=hsb[v][:])
                        ab2 = p0.tile([P, 2], f32, tag="ab2")
                        nc.vector.tensor_scalar_mul(
                            out=ab2[:], in0=hsb[v][:, 0:2],
                            scalar1=sc_t[:, v:v + 1])
                        m = v0 + v
                        nc.vector.tensor_copy(out=a_own[:, m:m + 1],
                                              in_=ab2[:, 0:1])
                        nc.vector.tensor_copy(out=b_own[:, m:m + 1],
                                              in_=ab2[:, 1:2])
                        nc.vector.tensor_copy(out=d_own[:, m:m + 1],
                                              in_=dba[:, v:v + 1])
                nc.gpsimd.dma_start(out=ccy_in_v[:, v0:v0 + nv, :],
                                    in_=y8[:, 0:nv, :])
                nc.sync.dma_start(
                    out=hn_own[v0 * P:(v0 + nv) * P, :].rearrange(
                        "(v p) e -> p v e", v=nv, p=P),
                    in_=hf[:, 0:nv, :])

        nc.vector.tensor_scalar_mul(out=dsc_t[:], in0=dsc_t[:], scalar1=1.0 / Y8)

        # AllGather the fp8 feature table
        if not skip_cc:
            nc.gpsimd.collective_compute(
                "AllGather", mybir.AluOpType.bypass,
                replica_groups=[list(range(NC))],
                ins=[ccy_in.opt()], outs=[ccy_out.opt()])
        else:
            nc.gpsimd.dma_start(out=ccy_out[0], in_=ccy_in[:])

        yrows = ccy_out[:].rearrange("r p (g e) -> (r p g) e", g=GPC, e=D)

        # ================= phase 3: full-dot aggregation =================
        with tc.tile_pool(name="p3c", bufs=1) as p3c, \
             tc.tile_pool(name="p3", bufs=2) as p3, \
             tc.tile_pool(name="p3ps", bufs=2, space="PSUM") as p3ps:
            srcl_all = p3c.tile([P, GPC, BT], f32)
            nc.sync.dma_start(out=srcl_all[:],
                              in_=srclT.rearrange("g p s -> p g s"))
            iota_i = p3c.tile([P, P], dt.int32)
            iota_f = p3c.tile([P, P], bf16)
            nc.gpsimd.iota(iota_i[:], pattern=[[1, P]], base=0,
                           channel_multiplier=0)
            nc.vector.tensor_copy(out=iota_f[:], in_=iota_i[:])
            n3 = GPC // G3 + (1 if GPC % G3 else 0)
            for ib in range(n3):
                g0 = ib * G3
                gn = min(G3, GPC - g0)
                tg = {}
                for y in (0, 1):
                    s = B[y] * 8
                    tidx = p3.tile([P, G3 * s], i16, tag=f"yi{y}")
                    nc.sync.dma_start(
                        out=tidx[:, 0:gn * s].rearrange("p (g s) -> p g s",
                                                        g=gn, s=s),
                        in_=yidxT[y][g0:g0 + gn].rearrange("g p s -> p g s"))
                    t = p3.tile([P, G3 * B[y], D], f8, tag=f"tg{y}")
                    nc.gpsimd.dma_gather(
                        out_ap=t[:, 0:gn * B[y], :],
                        in_ap=yrows[y * HALF_ROWS:(y + 1) * HALF_ROWS, :],
                        idxs_ap=tidx[:, 0:gn * s],
                        num_idxs=gn * B[y] * P,
                        num_idxs_reg=gn * B[y] * P, elem_size=D,
                        single_packet=False)
                    tg[y] = t
                for gi in range(gn):
                    g = g0 + gi
                    seq = build_seq(p3, g, iota_f, srcl_all)
                    seq8 = p3.tile([P, BT, P], f8, tag="seq8")
                    nc.scalar.activation(seq8[:], seq[:], AF.Copy)
                    pm = p3ps.tile([P, D], f32, tag="M", space="PSUM")
                    nmm = BT // 2
                    mi = 0
                    for y in (0, 1):
                        for db in range(B[y] // 2):
                            boff = (0 if y == 0 else B[0]) + 2 * db
                            j0 = gi * B[y] + 2 * db
                            nc.tensor.matmul(
                                out=pm[:],
                                lhsT=seq8[:, boff:boff + 2, :],
                                rhs=tg[y][:, j0:j0 + 2, :],
                                start=(mi == 0), stop=(mi == nmm - 1),
                                perf_mode=DR)
                            mi += 1
                    hsl = p3.tile([P, D], bf16, tag="hsl")
                    nc.sync.dma_start(out=hsl[:], in_=hn_own[g * P:(g + 1) * P, :])
                    hs = p3.tile([P, D], f32, tag="hs")
                    nc.scalar.activation(hs[:], hsl[:], AF.Copy,
                                         scale=dsc_t[:, g:g + 1])
                    scr = p3.tile([P, D], f32, tag="scr")
                    nc.vector.tensor_mul(out=scr[:], in0=pm[:], in1=hs[:])
                    nc.vector.reduce_sum(out=ang1[:, g:g + 1], in_=scr[:],
                                         axis=mybir.AxisListType.X)
                    hsum = p3.tile([P, 1], f32, tag="hsum")
                    nc.vector.reduce_sum(out=hsum[:], in_=scr[:, 0:2],
                                         axis=mybir.AxisListType.X)
                    nc.vector.tensor_sub(out=T_own[:, g:g + 1],
                                         in0=ang1[:, g:g + 1], in1=hsum[:])

        # ================= layers =================
        # Layers 2,3: P_i = sum_e (a*dinv)_dst and Q_i likewise are fetched
        # per edge by ONE GPSIMD ap_gather per half from a partition-
        # replicated [HALF_ROWS+1]-entry f32 table (bf16 lanes pack the
        # pair; last entry is zero for padding slots).  k-major list order
        # makes the diagonal mask (pG == p%16) periodic in 32 elements, so
        # one [P, Kmax*32] mask serves every per-(g, half) slot count K.
        # Consume per (g, half): masked mul (DVE 2x), S1 = accum of both
        # lanes (DVE 4x) = P+Q, S2 = accum of lane0 (Act) = P.
        with tc.tile_pool(name="lay", bufs=1) as lay:
            lidx_sb = lay.tile([P, max(NIh[0], NIh[1]) // 16], i16)
            lmask_sb = lay.tile([P, Kmax * 32], bf16)
            nc.sync.dma_start(out=lmask_sb[:], in_=lmaskT)
            ltab = lay.tile([P, HALF_ROWS + 1], f32)
            nc.gpsimd.memset(ltab[:, HALF_ROWS:HALF_ROWS + 1], 0.0)
            S1h = lay.tile([P, 2, GPC], f32)
            S2h = lay.tile([P, 2, GPC], f32)

            for layer in (1, 2, 3):
                if layer == 1:
                    ang_src = ang1
                else:
                    ccab_in = dram2.tile([P, GPC], f32, tag="ccab_in")
                    ccab_out = dram2.tile([NC, P, GPC], f32, tag="ccab_out")
                    upkv = r3[:].bitcast(bf16).rearrange(
                        "p (g l) -> p g l", l=2)
                    nc.vector.tensor_mul(out=r1[:], in0=a_own[:],
                                         in1=dinv_sb[:])
                    nc.vector.tensor_mul(out=r2[:], in0=b_own[:],
                                         in1=dinv_sb[:])
                    nc.vector.tensor_copy(out=upkv[:, :, 0:1],
                                          in_=r1[:, :, None])
                    nc.vector.tensor_copy(out=upkv[:, :, 1:2],
                                          in_=r2[:, :, None])
                    nc.gpsimd.dma_start(out=ccab_in[:], in_=r3[:])
                    if not skip_cc:
                        nc.gpsimd.collective_compute(
                            "AllGather", mybir.AluOpType.bypass,
                            replica_groups=[list(range(NC))],
                            ins=[ccab_in.opt()], outs=[ccab_out.opt()])
                    else:
                        nc.gpsimd.dma_start(out=ccab_out[0], in_=ccab_in[:])
                    with tc.tile_pool(name=f"l{layer}", bufs=1) as lp, \
                         tc.tile_pool(name=f"l{layer}s", bufs=2) as lps:
                        for y in (0, 1):
                            half = ccab_out[4 * y:4 * y + 4].rearrange(
                                "r p g -> (r p g)")
                            nc.sync.dma_start(
                                out=ltab[:, 0:HALF_ROWS],
                                in_=half.partition_broadcast(P))
                            nc.sync.dma_start(
                                out=lidx_sb[:, 0:NIh[y] // 16], in_=lidxT[y])
                            gout = lp.tile([P, NIh[y], 1], f32, tag="gout")
                            nc.gpsimd.ap_gather(
                                out_ap=gout[:],
                                in_ap=ltab[:, :, None],
                                idxs_ap=lidx_sb[:, 0:NIh[y] // 16],
                                channels=P, num_elems=HALF_ROWS + 1, d=1,
                                num_idxs=NIh[y])
                            gb = gout[:, :, 0].bitcast(bf16)
                            for g in range(GPC):
                                kg = int(Kg[g, y])
                                o0 = int(LOFF[y][g]) * 32
                                mm = lps.tile([P, Kmax * 32], bf16, tag="mm")
                                nc.vector.tensor_mul(
                                    out=mm[:, 0:kg * 32],
                                    in0=gb[:, o0:o0 + kg * 32],
                                    in1=lmask_sb[:, 0:kg * 32])
                                nc.vector.tensor_scalar(
                                    out=mm[:, 0:kg * 32], in0=mm[:, 0:kg * 32],
                                    scalar1=1.0, scalar2=0.0, op0=OP.mult,
                                    op1=OP.add,
                                    accum_out=S1h[:, y, g:g + 1])
                                alane = mm[:, 0:kg * 32].rearrange(
                                    "p (s l) -> p s l", l=2)[:, :, 0]
                                nc.scalar.activation(
                                    alane, alane, AF.Copy,
                                    accum_out=S2h[:, y, g:g + 1])
                    nc.vector.tensor_add(out=P_all[:], in0=S2h[:, 0],
                                         in1=S2h[:, 1])
                    nc.vector.tensor_add(out=Q_all[:], in0=S1h[:, 0],
                                         in1=S1h[:, 1])
                    nc.vector.tensor_sub(out=Q_all[:], in0=Q_all[:],
                                         in1=P_all[:])
                    nc.vector.tensor_mul(out=r1[:], in0=P_all[:],
                                         in1=a_own[:])
                    nc.vector.tensor_mul(out=r2[:], in0=Q_all[:],
                                         in1=b_own[:])
                    nc.vector.tensor_add(out=r1[:], in0=r1[:], in1=r2[:])
                    nc.vector.tensor_mul(out=r1[:], in0=r1[:],
                                         in1=dinv_sb[:])
                    nc.vector.tensor_add(out=angL[:], in0=T_own[:],
                                         in1=r1[:])
                    ang_src = angL
                nc.scalar.activation(c_t[:], ang_src[:], AF.Sin,
                                     bias=halfpi[:])
                nc.scalar.activation(s_t[:], ang_src[:], AF.Sin)
                nc.vector.tensor_mul(out=r1[:], in0=c_t[:], in1=a_own[:])
                nc.vector.tensor_mul(out=r2[:], in0=s_t[:], in1=b_own[:])
                nc.vector.tensor_mul(out=r3[:], in0=s_t[:], in1=a_own[:])
                nc.vector.tensor_mul(out=r4[:], in0=c_t[:], in1=b_own[:])
                nc.vector.tensor_sub(out=a_own[:], in0=r1[:], in1=r2[:])
                nc.vector.tensor_add(out=b_own[:], in0=r3[:], in1=r4[:])

        # ---- classifier constants + write final (a,b)*d into hn_own ----
        cls = stack.enter_context(tc.tile_pool(name="cls", bufs=1))
        cw1b = cls.tile([P, 4, D], bf16)
        cw2b = cls.tile([P, 4, DOUT], bf16)
        ident = cls.tile([P, P], f32)
        nc.gpsimd.dma_start(out=cw1b[:], in_=cW1.rearrange(
            "(k p) f -> p k f", k=4, p=P))
        nc.gpsimd.dma_start(out=cw2b[:], in_=cW2.rearrange(
            "(k p) f -> p k f", k=4, p=P))
        make_identity(nc, ident[:])
        nc.vector.tensor_mul(out=r1[:], in0=a_own[:], in1=d_own[:])
        nc.vector.tensor_mul(out=r2[:], in0=b_own[:], in1=d_own[:])
        abw = cls.tile([P, GPC, 2], bf16)
        hTall = cls.tile([P, 4, NPC], bf16)
        nc.vector.tensor_copy(out=abw[:, :, 0:1], in_=r1[:, :, None])
        nc.vector.tensor_copy(out=abw[:, :, 1:2], in_=r2[:, :, None])
        nc.sync.dma_start(
            out=hn_own.rearrange("(g p) e -> p g e", g=GPC, p=P)[:, :, 0:2],
            in_=abw[:])
        for k in range(4):
            nc.sync.dma_start_transpose(out=hTall[:, k, :],
                                        in_=hn_own[:, k * P:(k + 1) * P])

        # ================= phase 5: classifier (transposed domain) ========
        # BN affine is folded into cW2/cb2 host-side; Exp stays resident on
        # the Act engine (log-sum Ln runs once, batched, at the end).
        with tc.tile_pool(name="p5", bufs=3) as p5, \
             tc.tile_pool(name="p5c", bufs=1) as p5c, \
             tc.tile_pool(name="p5ps", bufs=2, space="PSUM") as p5ps:
            sh_all = p5c.tile([P, GPC, DOUT], f32)
            se_all = p5c.tile([P, GPC], f32)
            ls_all = p5c.tile([P, GPC], f32)
            for g in range(GPC):
                zTd = p5ps.tile([P, 4, P], f32, tag="zT", space="PSUM")
                for of in range(4):
                    for k in range(4):
                        nc.tensor.matmul(out=zTd[:, of, :],
                                         lhsT=cw1b[:, k, of * P:(of + 1) * P],
                                         rhs=hTall[:, k, g * P:(g + 1) * P],
                                         start=(k == 0), stop=(k == 3))
                zr = p5.tile([P, 4, P], bf16, tag="zr")
                if not FL.get("cb1_zero", True):
                    for of in range(4):
                        nc.vector.tensor_scalar(
                            out=zr[:, of, :], in0=zTd[:, of, :],
                            scalar1=cb1mT[:, of:of + 1], scalar2=0.0,
                            op0=OP.add, op1=OP.max)
                else:
                    nc.vector.tensor_scalar_max(out=zr[:], in0=zTd[:], scalar1=0.0)
                lgT = p5ps.tile([DOUT, P], f32, tag="lgps", space="PSUM")
                for k in range(4):
                    nc.tensor.matmul(out=lgT[:], lhsT=cw2b[:, k, :],
                                     rhs=zr[:, k, :],
                                     start=(k == 0), stop=(k == 3))
                lg_sb = p5.tile([DOUT, P], f32, tag="lgsb")
                nc.vector.tensor_copy(out=lg_sb[:], in_=lgT[:])
                ptr = p5ps.tile([P, DOUT], f32, tag="tr", space="PSUM")
                nc.tensor.transpose(out=ptr[:], in_=lg_sb[:],
                                    identity=ident[0:DOUT, 0:DOUT])
                lgv = p5.tile([P, DOUT], f32, tag="lgv")
                if not FL.get("cb2_zero", True):
                    nc.vector.tensor_add(out=lgv[:], in0=ptr[:], in1=cb2m[:])
                else:
                    nc.vector.tensor_copy(out=lgv[:], in_=ptr[:])
                mx = p5.tile([P, 1], f32, tag="mx")
                nc.vector.reduce_max(out=mx[:], in_=lgv[:],
                                     axis=mybir.AxisListType.X)
                nc.vector.tensor_scalar_sub(out=sh_all[:, g, :], in0=lgv[:],
                                            scalar1=mx[:])
                ex = p5.tile([P, DOUT], f32, tag="ex")
                nc.scalar.activation(ex[:], sh_all[:, g, :], AF.Exp,
                                     accum_out=se_all[:, g:g + 1])
            nc.scalar.activation(ls_all[:], se_all[:], AF.Ln)
            for g in range(GPC):
                ob = p5.tile([P, DOUT], f32, tag="ob")
                nc.vector.tensor_scalar_sub(out=ob[:], in0=sh_all[:, g, :],
                                            scalar1=ls_all[:, g:g + 1])
                nc.sync.dma_start(out=out[g * P:(g + 1) * P, :], in_=ob[:])

    nc.compile()
    return nc


# ---------------------------------------------------------------- in_maps

def make_in_maps(cfg, percore, weights):
    ins = []
    for r in range(cfg.NC):
        pc = percore[r]
        m = dict(
            xT=pc["xT"],
            W_in=weights["W_in"], b_in=weights["b_in"][None, :],
            ln_g=weights["ln_g"][None, :], ln_b=weights["ln_b"][None, :],
            cW1=weights["cW1"], cb1=weights["cb1"][None, :],
            bn_g=weights["bn_g"][None, :], bn_b=weights["bn_b"][None, :],
            bn_m=weights["bn_mean"][None, :], bn_v=weights["bn_var"][None, :],
            cW2=weights["cW2"], cb2=weights["cb2"][None, :],
            srcl=pc["srcl"], dinv=pc["dinv"],
            yidx0=pc["yidx0"], yidx1=pc["yidx1"],
            lidx0=pc["lidx0"], lidx1=pc["lidx1"], lmask=pc["lmask"],
        )
        ins.append(m)
    return ins


def assemble_output(cfg, results, n):
    chunks = [results[r]["out"] for r in range(cfg.NC)]
    full = np.concatenate(chunks, axis=0)
    return full[:n]


# ---------------------------------------------------------------- entry point

def _fold_bn(w):
    """Fold the eval-mode BatchNorm affine into cW2 / cb2 (host-side)."""
    w = dict(w)
    alpha = (w["bn_g"] / np.sqrt(w["bn_var"] + BN_EPS)).astype(np.float64)
    beta = w["bn_b"] - w["bn_mean"] * alpha
    cb2 = beta.astype(np.float64) @ w["cW2"].astype(np.float64) + w["cb2"]
    w["cW2"] = (alpha[:, None] * w["cW2"]).astype(np.float32)
    w["cb2"] = cb2.astype(np.float32)
    return w


def _cfg_flags(w):
    return dict(
        bin_zero=bool(np.all(w["b_in"] == 0)),
        ln_trivial=bool(np.all(w["ln_g"] == 1) and np.all(w["ln_b"] == 0)),
        cb1_zero=bool(np.all(w["cb1"] == 0)),
        cb2_zero=bool(np.all(w["cb2"] == 0)),
    )


def kernel(**inputs):
    """Full-input GNN forward on 8 TRN2 NeuronCores; returns [N, 40] fp32."""
    x = np.asarray(inputs["x"], np.float32)
    edge_src = np.asarray(inputs["edge_src"])
    edge_dst = np.asarray(inputs["edge_dst"])
    w = {k: np.asarray(inputs[k], np.float32) for k in
         ["W_in", "b_in", "ln_g", "ln_b", "cW1", "cb1", "bn_g", "bn_b",
          "bn_mean", "bn_var", "cW2", "cb2"]}
    N = x.shape[0]

    w = _fold_bn(w)
    cfg, percore = host_prep(x, edge_src, edge_dst, n_cores=8)
    cfg.flags = _cfg_flags(w)
    nc = build_nc(cfg)
    in_maps = make_in_maps(cfg, percore, w)

    from concourse.bass_utils import run_bass_kernel_spmd
    res = run_bass_kernel_spmd(nc, in_maps, core_ids=list(range(cfg.NC)))
    return assemble_output(cfg, res.results, N).astype(np.float32)


def estimate_exec_ns(inputs):
    """Tile cost-model (TimelineSim) estimate of the per-core program span.

    Collective latencies are excluded (replaced by local shard copies to
    preserve the dependency structure); everything else is modeled."""
    x = np.asarray(inputs["x"], np.float32)
    w = {k: np.asarray(inputs[k], np.float32) for k in
         ["W_in", "b_in", "ln_g", "ln_b", "cW1", "cb1", "bn_g", "bn_b",
          "bn_mean", "bn_var", "cW2", "cb2"]}
    w = _fold_bn(w)
    cfg, _ = host_prep(x, np.asarray(inputs["edge_src"]),
                       np.asarray(inputs["edge_dst"]), n_cores=8)
    cfg.flags = _cfg_flags(w)
    nc2 = build_nc(cfg, skip_cc=True)
    from concourse.timeline_sim import TimelineSim
    tl = TimelineSim(nc2, trace=False)
    ns = tl.simulate()
    return int(ns)

